# revision 19
# baseline (speedup 1.0000x reference)
"""Trainium2 Bass kernel for nn_DepthRenderer (superquadric depth renderer).

Sharding: rows round-robin over 8 cores (core c owns image rows r = 8*lr+c,
lr=0..44).  Per-core layout [128 lanes, 45 lrows, 5 xblocks]; lane = x%128,
xblock = x//128.  Each core renders all 8 SQs (constants baked as immediates
into one SPMD program) and min-accumulates depth on device; host concatenates.

Sparsity: a SQ can only influence pixels where the ray enters its bounding
sphere: h(d) = (b.d)^2 - (C-3) * d^T A d > 0 (homogeneous quadratic in the ray
direction, so normalization-free).  The host evaluates h on a coarse pixel
subgrid, takes the bounding rectangle (+margin, rows rounded to multiples of 8
so the rect is the SAME static view on every core), and the device program
processes only that rect per SQ (~4.7x less work).  Pixels outside the rect
keep depth FAR; rect pixels use the exact in-rect mask, and the premask
boundary is depth-continuous (grazing rays integrate to ~FAR), so the coarse
rect is safe.

Math notes (exact rewrites of the reference, up to fp rounding):
  - a == sizes  =>  X = |loc|/a + eps = |pts_loc| + eps  (sizes cancel)
  - ||td * sizes|| = ||d|| * rinv  (rotation invariance)
  - dt0  = ||pts_loc[0]*s + R^T p||,  dt10 = ||(PL10-PL9)*s||  with
    PL10 = loc_far/s = 1.5*u - (R^T p)/s
  - sqrt(x) = exp(0.5*ln(x)); sigmoid(x) = 0.5 + 0.5*tanh(x/2)
  - phase 1 (pow chains) uses the natural_log_exp ACT table set, phase 2
    (tanh occupancy + visibility exp) uses exp_and_others; both loads are
    pre-placed so bacc inserts no further table switches.
"""

from contextlib import ExitStack

import numpy as np

import concourse.bass as bass
import concourse.bacc as bacc
import concourse.mybir as mybir
from concourse import tile
from concourse.bass_utils import run_bass_kernel_spmd

F32 = mybir.dt.float32
AF = mybir.ActivationFunctionType
OP = mybir.AluOpType

# renderer constants (match the nn.Module init)
HS, WS = 360, 640
NEAR, FAR = 0.0, 1.5
NS = 10
SHARP = 1000.0
TAU = 100.0
N_SQ = 8
EPS = 1e-6

N_CORES = 8
NRL = HS // N_CORES       # 45 local rows per core
NJ = WS // 128            # 5 x-blocks
NCOL = NRL * NJ           # 225 columns per core
P = 128


def _f(x):
    return float(np.float32(x))


def _host_consts(sq_poses, sq_params, rays_o, t):
    """Per-SQ scalars, computed in float64 from the f32 inputs."""
    sq_poses = np.asarray(sq_poses, np.float64)
    sq_params = np.asarray(sq_params, np.float64)
    rays_o = np.asarray(rays_o, np.float64)
    t = np.asarray(t, np.float64)

    consts = []
    for k in range(N_SQ):
        R = sq_poses[k, :3, :3]
        p = sq_poses[k, :3, 3]
        s = sq_params[k, 0:3]
        e1 = sq_params[k, 3]
        e2 = sq_params[k, 4]

        M1 = R.T / s[:, None]            # u = M1 @ d = (R^T d)/s
        tc = (R.T @ (rays_o - p)) / s
        rp = R.T @ p                      # loc(near) = -rp
        rps = rp / s
        c1 = 2.0 / e2
        c2 = e2 / e1
        c3 = 2.0 / e1

        # near-point occupancy (constant per SQ)
        Xn = np.abs(-rp) / s + EPS
        fN = (Xn[0] ** c1 + Xn[1] ** c1) ** c2 + Xn[2] ** c3
        Fn = fN ** e1
        with np.errstate(over="ignore"):
            occ0 = 1.0 / (1.0 + np.exp(-SHARP * (1.0 - Fn)))
        vis0 = np.exp(-TAU * occ0)

        consts.append(dict(
            M1=M1, tc=tc, rp=rp, rps=rps, s=s,
            c1=c1, c2=c2, c3=c3, e1=e1,
            occ0=occ0, vis0=vis0,
        ))

    # segment weights from t (shared across SQs)
    dt_abs = np.abs(np.diff(t))          # |t_i - t_{i-1}|, i=1..9
    beta = np.zeros(11)                  # weight of v_s (s=1..10) in inner sum
    for i in range(1, NS):               # inner gaps i=1..9 use v_i, v_{i+1}
        beta[i] += 0.5 * dt_abs[i - 1]
        beta[i + 1] += 0.5 * dt_abs[i - 1]
    return consts, t, beta


def _host_rects(consts, rays_d):
    """Per-SQ (lr0, nr, j0, nj) bounding rect, identical across cores.

    h(d) = (b.d)^2 - (C-3) d^T A d is degree-2 homogeneous in d, so the
    coarse-subgrid sign test needs no ray normalization.  Conservative by a
    9px margin (>> 3px grid step; min blob diameter is ~40px for any SQ with
    C comfortably > 3).  Rows rounded to multiples of 8 so that every core's
    local-row range is the same [lr0, lr0+nr).
    """
    d = np.asarray(rays_d, np.float64)
    ys = np.arange(0, HS, 3)
    xs = np.arange(0, WS, 3)
    sub = d[np.ix_(ys, xs)]
    rects = []
    for cc in consts:
        M1, tcv = cc["M1"], cc["tc"]
        C = float((tcv ** 2).sum())
        if C <= 3.5:                      # near/inside bounding sphere: dense
            rects.append((0, NRL, 0, NJ))
            continue
        A = M1.T @ M1
        b = M1.T @ tcv
        hq = (sub @ b) ** 2 - (C - 3.0) * np.einsum("yxi,ij,yxj->yx", sub, A, sub)
        hit = hq > 0
        if not hit.any():
            rects.append(None)
            continue
        ryy, rxx = np.where(hit)
        r0 = max(0, int(ys[ryy.min()]) - 9)
        r1 = min(HS - 1, int(ys[ryy.max()]) + 9)
        x0 = max(0, int(xs[rxx.min()]) - 9)
        x1 = min(WS - 1, int(xs[rxx.max()]) + 9)
        r0 = (r0 // 8) * 8
        r1 = min(HS, ((r1 + 8) // 8) * 8) - 1
        lr0, nr = r0 // 8, (r1 - r0 + 1) // 8
        j0, j1 = x0 // 128, x1 // 128
        rects.append((lr0, nr, j0, j1 - j0 + 1))
    return rects


def build_program(consts, t, beta, rects, act_loads=True):
    """One SPMD program; input rdin [128,3,45,5], output depth [128,45,5]."""
    nc = bacc.Bacc("TRN2", target_bir_lowering=False, debug=False,
                   enable_asserts=False, num_devices=N_CORES)

    rd_dram = nc.dram_tensor("rdin", [P, 3, NRL, NJ], F32, kind="ExternalInput")
    out_dram = nc.dram_tensor("depth", [P, NRL, NJ], F32, kind="ExternalOutput")

    # const APs for activation biases (only 0.0/1.0 are pre-registered)
    def reg_const(v):
        v = _f(v)
        if (F32, v) not in nc.const_aps.aps:
            th = nc.alloc_sbuf_tensor(f"constap{len(nc.const_aps.aps)}", [128, 1], F32)
            nc.gpsimd.memset(th.ap(), v)
            nc.const_aps.aps[(F32, v)] = th.ap()

    reg_const(EPS)
    reg_const(SHARP / 2.0)
    for cc in consts:
        for j in range(3):
            reg_const(cc["rp"][j])
    nc.all_engine_barrier()

    live = [k for k in range(N_SQ) if rects[k] is not None]

    with tile.TileContext(nc) as tc, ExitStack() as es:
        V = nc.vector
        S = nc.scalar
        persist = es.enter_context(tc.tile_pool(name="persist", bufs=1))

        # ---- shared loads & per-core shared prep ----
        rd = persist.tile([P, 3, NRL, NJ], F32, name="rd")
        nc.sync.dma_start(rd[:, :, :, :], rd_dram.ap())

        rdsq = persist.tile([P, 3, NRL, NJ], F32, name="rdsq")
        S.activation(rdsq[:, :, :, :], rd[:, :, :, :], AF.Square)
        nd2 = persist.tile([P, NRL, NJ], F32, name="nd2")
        V.tensor_tensor(nd2[:, :, :], rdsq[:, 0, :, :], rdsq[:, 1, :, :], OP.add)
        V.tensor_tensor(nd2[:, :, :], nd2[:, :, :], rdsq[:, 2, :, :], OP.add)
        nd = persist.tile([P, NRL, NJ], F32, name="nd")
        S.activation(nd[:, :, :], nd2[:, :, :], AF.Ln)
        S.activation(nd[:, :, :], nd[:, :, :], AF.Exp, scale=0.5)

        dmin = persist.tile([P, NRL, NJ], F32, name="dmin")
        V.memset(dmin[:, :, :], FAR)

        XMAX = max(r[1] * r[3] for r in rects if r is not None)
        gate = persist.tile([P, XMAX, NS + 1], F32, name="gate")
        V.memset(gate[:, :, :], 1.0)
        V.memset(gate[:, :, 0], 0.0)
        betaT = persist.tile([P, XMAX, NS + 1], F32, name="betaT")
        for si in range(NS):
            V.memset(betaT[:, :, si], _f(beta[si + 1]))
        V.memset(betaT[:, :, NS], 0.0)

        # persistent per-SQ results for phase 2 (sized per rect)
        FF, MK, HG, DTT = {}, {}, {}, {}
        for k in live:
            lr0, nr, j0, nj = rects[k]
            X = nr * nj
            FF[k] = persist.tile([P, X, NS + 1], F32, name=f"FF{k}")
            MK[k] = persist.tile([P, X], F32, name=f"MK{k}")
            HG[k] = persist.tile([P, X], F32, name=f"HG{k}")
            DTT[k] = persist.tile([P, 2, X], F32, name=f"DTT_{k}")

        # ---------------- phase 1: per-SQ F chains (ln/exp table set) -------
        with tc.tile_pool(name="p1", bufs=3) as pool:
            for k in live:
                cc = consts[k]
                E = V
                lr0, nr, j0, nj = rects[k]
                X = nr * nj
                M1, tcv, rp, rps, s = cc["M1"], cc["tc"], cc["rp"], cc["rps"], cc["s"]

                def r4(ap2):   # [P, X] compact view -> [P, nr, nj]
                    return ap2.rearrange("p (a b) -> p a b", b=nj)

                # compact copies of the rect slice of rd (and ||d||)
                rdc = pool.tile([P, 3, X], F32, tag="rdc")
                E.tensor_copy(rdc[:, :, :].rearrange("p c (a b) -> p c a b", b=nj),
                              rd[:, :, lr0:lr0 + nr, j0:j0 + nj])
                ndc = pool.tile([P, X], F32, tag="ndc")
                E.tensor_copy(r4(ndc[:, :]), nd[:, lr0:lr0 + nr, j0:j0 + nj])

                u = pool.tile([P, 3, X], F32, tag="u")
                for j in range(3):
                    E.tensor_scalar(u[:, j, :], rdc[:, 0, :], _f(M1[j, 0]), None, OP.mult)
                    E.scalar_tensor_tensor(u[:, j, :], rdc[:, 1, :], _f(M1[j, 1]), u[:, j, :], OP.mult, OP.add)
                    E.scalar_tensor_tensor(u[:, j, :], rdc[:, 2, :], _f(M1[j, 2]), u[:, j, :], OP.mult, OP.add)

                usq = pool.tile([P, 3, X], F32, tag="usq")
                E.tensor_tensor(usq[:, :, :], u[:, :, :], u[:, :, :], OP.mult)
                nu2 = pool.tile([P, X], F32, tag="nu2")
                E.tensor_tensor(nu2[:], usq[:, 0, :], usq[:, 1, :], OP.add)
                E.tensor_tensor(nu2[:], nu2[:], usq[:, 2, :], OP.add)

                rinv = pool.tile([P, X], F32, tag="rinv")
                S.activation(rinv[:], nu2[:], AF.Ln)
                S.activation(rinv[:], rinv[:], AF.Exp, scale=-0.5)

                td = pool.tile([P, 3, X], F32, tag="td")
                for j in range(3):
                    E.tensor_tensor(td[:, j, :], u[:, j, :], rinv[:], OP.mult)

                d1 = pool.tile([P, X], F32, tag="d1")
                E.tensor_scalar(d1[:], td[:, 0, :], _f(tcv[0]), None, OP.mult)
                E.scalar_tensor_tensor(d1[:], td[:, 1, :], _f(tcv[1]), d1[:], OP.mult, OP.add)
                E.scalar_tensor_tensor(d1[:], td[:, 2, :], _f(tcv[2]), d1[:], OP.mult, OP.add)
                proj = pool.tile([P, X], F32, tag="proj")
                S.activation(proj[:], d1[:], AF.Abs)

                cen = pool.tile([P, 3, X], F32, tag="cen")
                for j in range(3):
                    E.tensor_tensor(cen[:, j, :], proj[:], td[:, j, :], OP.mult)
                    E.tensor_scalar(cen[:, j, :], cen[:, j, :], _f(tcv[j]), None, OP.add)

                csq = pool.tile([P, 3, X], F32, tag="usq")
                E.tensor_tensor(csq[:, :, :], cen[:, :, :], cen[:, :, :], OP.mult)
                m3 = pool.tile([P, X], F32, tag="m3")
                E.tensor_tensor(m3[:], csq[:, 0, :], csq[:, 1, :], OP.add)
                E.tensor_tensor(m3[:], m3[:], csq[:, 2, :], OP.add)
                # m3 = 3 - dist^2 ; mask = m3 > 0 ; hclsq = max(m3, 1e-12)
                E.tensor_scalar(m3[:], m3[:], -1.0, 3.0, OP.mult, OP.add)
                E.tensor_scalar(MK[k][:], m3[:], 0.0, None, OP.is_gt)
                E.tensor_scalar(m3[:], m3[:], 1e-12, None, OP.max)

                hcl = pool.tile([P, X], F32, tag="hcl")
                S.activation(hcl[:], m3[:], AF.Ln)
                S.activation(hcl[:], hcl[:], AF.Exp, scale=0.5)

                # hg = hcl * ||d|| * rinv
                E.tensor_tensor(HG[k][:], ndc[:], rinv[:], OP.mult)
                E.tensor_tensor(HG[k][:], HG[k][:], hcl[:], OP.mult)

                htd = pool.tile([P, 3, X], F32, tag="htd")
                for j in range(3):
                    E.tensor_tensor(htd[:, j, :], hcl[:], td[:, j, :], OP.mult)

                # PL slots 0..9: cen + t_s*htd ; slot 10: 1.5*u - rp/s
                PL = pool.tile([P, NS + 1, 3, X], F32, tag="PL", bufs=3)
                for si in range(NS):
                    E.scalar_tensor_tensor(PL[:, si, :, :], htd[:, :, :], _f(t[si]),
                                           cen[:, :, :], OP.mult, OP.add)
                for j in range(3):
                    E.tensor_scalar(PL[:, NS, j, :], u[:, j, :], 1.5, _f(-rps[j]),
                                    OP.mult, OP.add)

                # dt0 = ||PL0*s + rp|| ; dt10 = ||(PL10-PL9)*s||
                q3 = pool.tile([P, 3, X], F32, tag="q3")
                for j in range(3):
                    S.activation(q3[:, j, :], PL[:, 0, j, :], AF.Square,
                                 bias=_f(rp[j]), scale=_f(s[j]))
                dtt = DTT[k]
                E.tensor_tensor(dtt[:, 0, :], q3[:, 0, :], q3[:, 1, :], OP.add)
                E.tensor_tensor(dtt[:, 0, :], dtt[:, 0, :], q3[:, 2, :], OP.add)

                df = pool.tile([P, 3, X], F32, tag="q3b")
                E.tensor_tensor(df[:, :, :], PL[:, NS, :, :], PL[:, NS - 1, :, :], OP.subtract)
                for j in range(3):
                    S.activation(df[:, j, :], df[:, j, :], AF.Square, scale=_f(s[j]))
                E.tensor_tensor(dtt[:, 1, :], df[:, 0, :], df[:, 1, :], OP.add)
                E.tensor_tensor(dtt[:, 1, :], dtt[:, 1, :], df[:, 2, :], OP.add)
                S.activation(dtt[:, :, :], dtt[:, :, :], AF.Ln)
                S.activation(dtt[:, :, :], dtt[:, :, :], AF.Exp, scale=0.5)

                # F chain, in place over PL
                flat = PL[:, :, :, :]
                S.activation(flat, flat, AF.Abs)                       # |PL|
                S.activation(flat, flat, AF.Ln, bias=_f(EPS))          # ln(|PL|+eps)
                S.activation(PL[:, :, 0:2, :], PL[:, :, 0:2, :], AF.Exp,
                             scale=_f(cc["c1"]))                       # u,v
                E.tensor_tensor(PL[:, :, 0, :], PL[:, :, 0, :], PL[:, :, 1, :], OP.add)
                S.activation(PL[:, :, 0, :], PL[:, :, 0, :], AF.Ln)
                S.activation(PL[:, :, 0, :], PL[:, :, 0, :], AF.Exp, scale=_f(cc["c2"]))
                S.activation(PL[:, :, 2, :], PL[:, :, 2, :], AF.Exp, scale=_f(cc["c3"]))
                E.tensor_tensor(PL[:, :, 0, :], PL[:, :, 0, :], PL[:, :, 2, :], OP.add)
                S.activation(PL[:, :, 0, :], PL[:, :, 0, :], AF.Ln)
                S.activation(FF[k][:, :, :].rearrange("p x s -> p s x"),
             PL[:, :, 0, :], AF.Exp, scale=_f(cc["e1"]))

        tc.no_sync_barrier()

        # ---------------- phase 2: occupancy/visibility (exp set) -----------
        # s-innermost layout [P, X, 11]: the occupancy cumsum and the
        # beta-weighted visibility sum each become ONE gated tensor_tensor_scan
        # (state = gate*state + data; gate=0 at s=0 resets per pixel).
        with tc.tile_pool(name="p2", bufs=3) as pool:
            for k in live:
                cc = consts[k]
                E = V
                lr0, nr, j0, nj = rects[k]
                X = nr * nj
                NSS = NS + 1
                g2 = gate[:, 0:X, :].rearrange("p a b -> p (a b)")
                th = pool.tile([P, X, NSS], F32, tag="th", bufs=3)
                # occ = 0.5 + 0.5*tanh(500*(1-F)) == sigmoid(1000*(1-F))
                S.activation(th[:, :, :], FF[k][:, :, :], AF.Tanh,
                             bias=SHARP / 2.0, scale=-SHARP / 2.0)
                E.tensor_scalar(th[:, :, :], th[:, :, :], 0.5, 0.5, OP.mult, OP.add)
                E.tensor_scalar(th[:, :, 0], th[:, :, 0], _f(cc["occ0"]), None, OP.add)

                cum = pool.tile([P, X, NSS], F32, tag="cum", bufs=3)
                E.tensor_tensor_scan(cum[:, :, :].rearrange("p a b -> p (a b)"),
                                     g2, th[:, :, :].rearrange("p a b -> p (a b)"),
                                     0.0, OP.mult, OP.add)
                S.activation(cum[:, :, :], cum[:, :, :], AF.Exp, scale=-TAU)  # vis

                wv = pool.tile([P, X, NSS], F32, tag="wv")
                E.tensor_tensor(wv[:, :, :], cum[:, :, :], betaT[:, 0:X, :], OP.mult)
                E.tensor_tensor_scan(wv[:, :, :].rearrange("p a b -> p (a b)"),
                                     g2, wv[:, :, :].rearrange("p a b -> p (a b)"),
                                     0.0, OP.mult, OP.add)

                acc = pool.tile([P, X], F32, tag="acc")
                E.tensor_tensor(acc[:], wv[:, :, NS], HG[k][:], OP.mult)

                b1 = pool.tile([P, X], F32, tag="b1")
                E.tensor_scalar(b1[:], cum[:, :, 0], 0.5, _f(0.5 * cc["vis0"]),
                                OP.mult, OP.add)
                E.tensor_tensor(b1[:], b1[:], DTT[k][:, 0, :], OP.mult)
                E.tensor_tensor(acc[:], acc[:], b1[:], OP.add)

                b2 = pool.tile([P, X], F32, tag="b2")
                E.tensor_tensor(b2[:], cum[:, :, NS - 1], cum[:, :, NS], OP.add)
                E.scalar_tensor_tensor(b2[:], b2[:], 0.5, DTT[k][:, 1, :], OP.mult, OP.mult)
                E.tensor_tensor(acc[:], acc[:], b2[:], OP.add)

                # dmin[rect] = min(dmin[rect], mask ? depth : FAR)
                E.tensor_scalar(acc[:], acc[:], 1.0, -FAR, OP.mult, OP.add)
                E.tensor_tensor(acc[:], acc[:], MK[k][:], OP.mult)
                E.tensor_scalar(acc[:], acc[:], FAR, None, OP.add)
                dv = dmin[:, lr0:lr0 + nr, j0:j0 + nj]
                V.tensor_tensor(dv, dv, acc[:].rearrange("p (a b) -> p a b", b=nj),
                                OP.min)

        nc.sync.dma_start(out_dram.ap(), dmin[:, :, :])

    # Pre-place the two ACT table loads (natural_log_exp for phase 1,
    # exp_and_others for phase 2/tanh) so bacc's fixpoint inserts none.
    # (CoreSim can't handle the hand-inserted loads; act_loads=False skips.)
    if not act_loads:
        nc.compile()
        return nc
    from concourse.hw_specs import get_activation_tables
    names = list(get_activation_tables(nc.m.arch).keys())
    id_nle = names.index("natural_log_exp_and_others")
    id_exp = names.index("exp_and_others")

    def make_load(set_id):
        ins = mybir.InstLoadActFuncSet(
            name=nc.get_next_instruction_name(), act_func_set_id=set_id,
            ins=[], outs=[])
        ins.engine = nc.scalar.engine
        return ins

    for blk in nc.main_func.blocks:
        il = blk.instructions
        first_act = next((i for i, x in enumerate(il)
                          if isinstance(x, mybir.InstActivation)), None)
        if first_act is None:
            continue
        first_tanh = next((i for i, x in enumerate(il)
                           if isinstance(x, mybir.InstActivation)
                           and x.func == AF.Tanh), None)
        il.insert(first_act, make_load(id_nle))
        if first_tanh is not None:
            il.insert(first_tanh + 1, make_load(id_exp))

    nc.compile()
    return nc


def _shard_rays(rays_d):
    """-> per-core arrays [128, 3, 45, 5]; core c owns rows 8*lr+c."""
    rd = np.asarray(rays_d, np.float32)
    out = []
    for c in range(N_CORES):
        sub = rd[c::N_CORES]                         # (45, 640, 3)
        arr = sub.reshape(NRL, NJ, 128, 3).transpose(2, 3, 0, 1)
        out.append(np.ascontiguousarray(arr))        # (128, 3, 45, 5)
    return out


def _unshard(outs):
    """outs: list of 8 arrays [128, 45, 5] -> (360, 640)."""
    full = np.empty((HS, WS), np.float32)
    for c in range(N_CORES):
        full[c::N_CORES] = outs[c].transpose(1, 2, 0).reshape(NRL, WS)
    return full


def kernel(sq_poses, sq_params, rays_d, rays_o, t, **run_kwargs):
    consts, tv, beta = _host_consts(sq_poses, sq_params, rays_o, t)
    rects = _host_rects(consts, rays_d)
    nc = build_program(consts, tv, beta, rects)
    planes = _shard_rays(rays_d)
    in_maps = [{"rdin": planes[c]} for c in range(N_CORES)]
    res = run_bass_kernel_spmd(nc, in_maps, core_ids=list(range(N_CORES)), **run_kwargs)
    outs = [res.results[c]["depth"] for c in range(N_CORES)]
    out = _unshard(outs).astype(np.float32)
    kernel.last_result = res
    return out


kernel.last_result = None


# revision 20
# speedup vs baseline: 1.0012x; 1.0012x over previous
"""Trainium2 Bass kernel for nn_DepthRenderer (superquadric depth renderer).

Sharding: rows round-robin over 8 cores (core c owns image rows r = 8*lr+c,
lr=0..44).  Per-core layout [128 lanes, 45 lrows, 5 xblocks]; lane = x%128,
xblock = x//128.  Each core renders all 8 SQs (constants baked as immediates
into one SPMD program) and min-accumulates depth on device; host concatenates.

Sparsity: a SQ can only influence pixels where the ray enters its bounding
sphere: h(d) = (b.d)^2 - (C-3) * d^T A d > 0 (homogeneous quadratic in the ray
direction, so normalization-free).  The host evaluates h on a coarse pixel
subgrid, takes the bounding rectangle (+margin, rows rounded to multiples of 8
so the rect is the SAME static view on every core), and the device program
processes only that rect per SQ (~4.7x less work).  Pixels outside the rect
keep depth FAR; rect pixels use the exact in-rect mask, and the premask
boundary is depth-continuous (grazing rays integrate to ~FAR), so the coarse
rect is safe.

Math notes (exact rewrites of the reference, up to fp rounding):
  - a == sizes  =>  X = |loc|/a + eps = |pts_loc| + eps  (sizes cancel)
  - ||td * sizes|| = ||d|| * rinv  (rotation invariance)
  - dt0  = ||pts_loc[0]*s + R^T p||,  dt10 = ||(PL10-PL9)*s||  with
    PL10 = loc_far/s = 1.5*u - (R^T p)/s
  - sqrt(x) = exp(0.5*ln(x)); sigmoid(x) = 0.5 + 0.5*tanh(x/2)
  - phase 1 (pow chains) uses the natural_log_exp ACT table set, phase 2
    (tanh occupancy + visibility exp) uses exp_and_others; both loads are
    pre-placed so bacc inserts no further table switches.
"""

from contextlib import ExitStack

import numpy as np

import concourse.bass as bass
import concourse.bacc as bacc
import concourse.mybir as mybir
from concourse import tile
from concourse.bass_utils import run_bass_kernel_spmd

F32 = mybir.dt.float32
AF = mybir.ActivationFunctionType
OP = mybir.AluOpType

# renderer constants (match the nn.Module init)
HS, WS = 360, 640
NEAR, FAR = 0.0, 1.5
NS = 10
SHARP = 1000.0
TAU = 100.0
N_SQ = 8
EPS = 1e-6

N_CORES = 8
NRL = HS // N_CORES       # 45 local rows per core
NJ = WS // 128            # 5 x-blocks
NCOL = NRL * NJ           # 225 columns per core
P = 128


def _f(x):
    return float(np.float32(x))


def _host_consts(sq_poses, sq_params, rays_o, t):
    """Per-SQ scalars, computed in float64 from the f32 inputs."""
    sq_poses = np.asarray(sq_poses, np.float64)
    sq_params = np.asarray(sq_params, np.float64)
    rays_o = np.asarray(rays_o, np.float64)
    t = np.asarray(t, np.float64)

    consts = []
    for k in range(N_SQ):
        R = sq_poses[k, :3, :3]
        p = sq_poses[k, :3, 3]
        s = sq_params[k, 0:3]
        e1 = sq_params[k, 3]
        e2 = sq_params[k, 4]

        M1 = R.T / s[:, None]            # u = M1 @ d = (R^T d)/s
        tc = (R.T @ (rays_o - p)) / s
        rp = R.T @ p                      # loc(near) = -rp
        rps = rp / s
        c1 = 2.0 / e2
        c2 = e2 / e1
        c3 = 2.0 / e1

        # near-point occupancy (constant per SQ)
        Xn = np.abs(-rp) / s + EPS
        fN = (Xn[0] ** c1 + Xn[1] ** c1) ** c2 + Xn[2] ** c3
        Fn = fN ** e1
        with np.errstate(over="ignore"):
            occ0 = 1.0 / (1.0 + np.exp(-SHARP * (1.0 - Fn)))
        vis0 = np.exp(-TAU * occ0)

        consts.append(dict(
            M1=M1, tc=tc, rp=rp, rps=rps, s=s,
            c1=c1, c2=c2, c3=c3, e1=e1,
            occ0=occ0, vis0=vis0,
        ))

    # segment weights from t (shared across SQs)
    dt_abs = np.abs(np.diff(t))          # |t_i - t_{i-1}|, i=1..9
    beta = np.zeros(11)                  # weight of v_s (s=1..10) in inner sum
    for i in range(1, NS):               # inner gaps i=1..9 use v_i, v_{i+1}
        beta[i] += 0.5 * dt_abs[i - 1]
        beta[i + 1] += 0.5 * dt_abs[i - 1]
    return consts, t, beta


def _host_rects(consts, rays_d):
    """Per-SQ (lr0, nr, j0, nj) bounding rect, identical across cores.

    h(d) = (b.d)^2 - (C-3) d^T A d is degree-2 homogeneous in d, so the
    coarse-subgrid sign test needs no ray normalization.  Conservative by a
    9px margin (>> 3px grid step; min blob diameter is ~40px for any SQ with
    C comfortably > 3).  Rows rounded to multiples of 8 so that every core's
    local-row range is the same [lr0, lr0+nr).
    """
    d = np.asarray(rays_d, np.float64)
    ys = np.arange(0, HS, 3)
    xs = np.arange(0, WS, 3)
    sub = d[np.ix_(ys, xs)]
    rects = []
    for cc in consts:
        M1, tcv = cc["M1"], cc["tc"]
        C = float((tcv ** 2).sum())
        if C <= 3.5:                      # near/inside bounding sphere: dense
            rects.append((0, NRL, 0, NJ))
            continue
        A = M1.T @ M1
        b = M1.T @ tcv
        hq = (sub @ b) ** 2 - (C - 3.0) * np.einsum("yxi,ij,yxj->yx", sub, A, sub)
        hit = hq > 0
        if not hit.any():
            rects.append(None)
            continue
        ryy, rxx = np.where(hit)
        r0 = max(0, int(ys[ryy.min()]) - 9)
        r1 = min(HS - 1, int(ys[ryy.max()]) + 9)
        x0 = max(0, int(xs[rxx.min()]) - 9)
        x1 = min(WS - 1, int(xs[rxx.max()]) + 9)
        r0 = (r0 // 8) * 8
        r1 = min(HS, ((r1 + 8) // 8) * 8) - 1
        lr0, nr = r0 // 8, (r1 - r0 + 1) // 8
        j0, j1 = x0 // 128, x1 // 128
        rects.append((lr0, nr, j0, j1 - j0 + 1))
    return rects


def build_program(consts, t, beta, rects, act_loads=True):
    """One SPMD program; input rdin [128,3,45,5], output depth [128,45,5]."""
    nc = bacc.Bacc("TRN2", target_bir_lowering=False, debug=False,
                   enable_asserts=False, num_devices=N_CORES)

    rd_dram = nc.dram_tensor("rdin", [P, 3, NRL, NJ], F32, kind="ExternalInput")
    out_dram = nc.dram_tensor("depth", [P, NRL, NJ], F32, kind="ExternalOutput")

    # const APs for activation biases (only 0.0/1.0 are pre-registered)
    def reg_const(v):
        v = _f(v)
        if (F32, v) not in nc.const_aps.aps:
            th = nc.alloc_sbuf_tensor(f"constap{len(nc.const_aps.aps)}", [128, 1], F32)
            nc.gpsimd.memset(th.ap(), v)
            nc.const_aps.aps[(F32, v)] = th.ap()

    reg_const(EPS)
    reg_const(SHARP / 2.0)
    for cc in consts:
        for j in range(3):
            reg_const(cc["rp"][j])
    nc.all_engine_barrier()

    live = [k for k in range(N_SQ) if rects[k] is not None]

    with tile.TileContext(nc) as tc, ExitStack() as es:
        V = nc.vector
        S = nc.scalar
        persist = es.enter_context(tc.tile_pool(name="persist", bufs=1))

        # ---- shared loads & per-core shared prep ----
        rd = persist.tile([P, 3, NRL, NJ], F32, name="rd")
        nc.sync.dma_start(rd[:, :, :, :], rd_dram.ap())

        rdsq = persist.tile([P, 3, NRL, NJ], F32, name="rdsq")
        S.activation(rdsq[:, :, :, :], rd[:, :, :, :], AF.Square)
        nd2 = persist.tile([P, NRL, NJ], F32, name="nd2")
        V.tensor_tensor(nd2[:, :, :], rdsq[:, 0, :, :], rdsq[:, 1, :, :], OP.add)
        V.tensor_tensor(nd2[:, :, :], nd2[:, :, :], rdsq[:, 2, :, :], OP.add)
        nd = persist.tile([P, NRL, NJ], F32, name="nd")
        S.activation(nd[:, :, :], nd2[:, :, :], AF.Ln)
        S.activation(nd[:, :, :], nd[:, :, :], AF.Exp, scale=0.5)

        dmin = persist.tile([P, NRL, NJ], F32, name="dmin")
        V.memset(dmin[:, :, :], FAR)

        XMAX = max(r[1] * r[3] for r in rects if r is not None)
        gate = persist.tile([P, XMAX, NS + 1], F32, name="gate")
        V.memset(gate[:, :, :], 1.0)
        V.memset(gate[:, :, 0], 0.0)
        betaT = persist.tile([P, XMAX, NS + 1], F32, name="betaT")
        for si in range(NS):
            V.memset(betaT[:, :, si], _f(beta[si + 1]))
        V.memset(betaT[:, :, NS], 0.0)

        # persistent per-SQ results for phase 2 (sized per rect)
        FF, MK, HG, DTT = {}, {}, {}, {}
        for k in live:
            lr0, nr, j0, nj = rects[k]
            X = nr * nj
            FF[k] = persist.tile([P, X, NS + 1], F32, name=f"FF{k}")
            MK[k] = persist.tile([P, X], F32, name=f"MK{k}")
            HG[k] = persist.tile([P, X], F32, name=f"HG{k}")
            DTT[k] = persist.tile([P, 2, X], F32, name=f"DTT_{k}")

        # ---------------- phase 1: per-SQ F chains (ln/exp table set) -------
        with tc.tile_pool(name="p1", bufs=6) as pool:
            for k in live:
                cc = consts[k]
                E = V
                lr0, nr, j0, nj = rects[k]
                X = nr * nj
                M1, tcv, rp, rps, s = cc["M1"], cc["tc"], cc["rp"], cc["rps"], cc["s"]

                def r4(ap2):   # [P, X] compact view -> [P, nr, nj]
                    return ap2.rearrange("p (a b) -> p a b", b=nj)

                # compact copies of the rect slice of rd (and ||d||)
                rdc = pool.tile([P, 3, X], F32, tag="rdc")
                E.tensor_copy(rdc[:, :, :].rearrange("p c (a b) -> p c a b", b=nj),
                              rd[:, :, lr0:lr0 + nr, j0:j0 + nj])
                ndc = pool.tile([P, X], F32, tag="ndc")
                E.tensor_copy(r4(ndc[:, :]), nd[:, lr0:lr0 + nr, j0:j0 + nj])

                u = pool.tile([P, 3, X], F32, tag="u")
                for j in range(3):
                    E.tensor_scalar(u[:, j, :], rdc[:, 0, :], _f(M1[j, 0]), None, OP.mult)
                    E.scalar_tensor_tensor(u[:, j, :], rdc[:, 1, :], _f(M1[j, 1]), u[:, j, :], OP.mult, OP.add)
                    E.scalar_tensor_tensor(u[:, j, :], rdc[:, 2, :], _f(M1[j, 2]), u[:, j, :], OP.mult, OP.add)

                usq = pool.tile([P, 3, X], F32, tag="usq")
                E.tensor_tensor(usq[:, :, :], u[:, :, :], u[:, :, :], OP.mult)
                nu2 = pool.tile([P, X], F32, tag="nu2")
                E.tensor_tensor(nu2[:], usq[:, 0, :], usq[:, 1, :], OP.add)
                E.tensor_tensor(nu2[:], nu2[:], usq[:, 2, :], OP.add)

                rinv = pool.tile([P, X], F32, tag="rinv")
                S.activation(rinv[:], nu2[:], AF.Ln)
                S.activation(rinv[:], rinv[:], AF.Exp, scale=-0.5)

                td = pool.tile([P, 3, X], F32, tag="td")
                for j in range(3):
                    E.tensor_tensor(td[:, j, :], u[:, j, :], rinv[:], OP.mult)

                d1 = pool.tile([P, X], F32, tag="d1")
                E.tensor_scalar(d1[:], td[:, 0, :], _f(tcv[0]), None, OP.mult)
                E.scalar_tensor_tensor(d1[:], td[:, 1, :], _f(tcv[1]), d1[:], OP.mult, OP.add)
                E.scalar_tensor_tensor(d1[:], td[:, 2, :], _f(tcv[2]), d1[:], OP.mult, OP.add)
                proj = pool.tile([P, X], F32, tag="proj")
                S.activation(proj[:], d1[:], AF.Abs)

                cen = pool.tile([P, 3, X], F32, tag="cen")
                for j in range(3):
                    E.tensor_tensor(cen[:, j, :], proj[:], td[:, j, :], OP.mult)
                    E.tensor_scalar(cen[:, j, :], cen[:, j, :], _f(tcv[j]), None, OP.add)

                csq = pool.tile([P, 3, X], F32, tag="usq")
                E.tensor_tensor(csq[:, :, :], cen[:, :, :], cen[:, :, :], OP.mult)
                m3 = pool.tile([P, X], F32, tag="m3")
                E.tensor_tensor(m3[:], csq[:, 0, :], csq[:, 1, :], OP.add)
                E.tensor_tensor(m3[:], m3[:], csq[:, 2, :], OP.add)
                # m3 = 3 - dist^2 ; mask = m3 > 0 ; hclsq = max(m3, 1e-12)
                E.tensor_scalar(m3[:], m3[:], -1.0, 3.0, OP.mult, OP.add)
                E.tensor_scalar(MK[k][:], m3[:], 0.0, None, OP.is_gt)
                E.tensor_scalar(m3[:], m3[:], 1e-12, None, OP.max)

                hcl = pool.tile([P, X], F32, tag="hcl")
                S.activation(hcl[:], m3[:], AF.Ln)
                S.activation(hcl[:], hcl[:], AF.Exp, scale=0.5)

                # hg = hcl * ||d|| * rinv
                E.tensor_tensor(HG[k][:], ndc[:], rinv[:], OP.mult)
                E.tensor_tensor(HG[k][:], HG[k][:], hcl[:], OP.mult)

                htd = pool.tile([P, 3, X], F32, tag="htd")
                for j in range(3):
                    E.tensor_tensor(htd[:, j, :], hcl[:], td[:, j, :], OP.mult)

                # PL slots 0..9: cen + t_s*htd ; slot 10: 1.5*u - rp/s
                PL = pool.tile([P, NS + 1, 3, X], F32, tag="PL", bufs=4)
                for si in range(NS):
                    E.scalar_tensor_tensor(PL[:, si, :, :], htd[:, :, :], _f(t[si]),
                                           cen[:, :, :], OP.mult, OP.add)
                for j in range(3):
                    E.tensor_scalar(PL[:, NS, j, :], u[:, j, :], 1.5, _f(-rps[j]),
                                    OP.mult, OP.add)

                # dt0 = ||PL0*s + rp|| ; dt10 = ||(PL10-PL9)*s||
                q3 = pool.tile([P, 3, X], F32, tag="q3")
                for j in range(3):
                    S.activation(q3[:, j, :], PL[:, 0, j, :], AF.Square,
                                 bias=_f(rp[j]), scale=_f(s[j]))
                dtt = DTT[k]
                E.tensor_tensor(dtt[:, 0, :], q3[:, 0, :], q3[:, 1, :], OP.add)
                E.tensor_tensor(dtt[:, 0, :], dtt[:, 0, :], q3[:, 2, :], OP.add)

                df = pool.tile([P, 3, X], F32, tag="q3b")
                E.tensor_tensor(df[:, :, :], PL[:, NS, :, :], PL[:, NS - 1, :, :], OP.subtract)
                for j in range(3):
                    S.activation(df[:, j, :], df[:, j, :], AF.Square, scale=_f(s[j]))
                E.tensor_tensor(dtt[:, 1, :], df[:, 0, :], df[:, 1, :], OP.add)
                E.tensor_tensor(dtt[:, 1, :], dtt[:, 1, :], df[:, 2, :], OP.add)
                S.activation(dtt[:, :, :], dtt[:, :, :], AF.Ln)
                S.activation(dtt[:, :, :], dtt[:, :, :], AF.Exp, scale=0.5)

                # F chain, in place over PL
                flat = PL[:, :, :, :]
                S.activation(flat, flat, AF.Abs)                       # |PL|
                S.activation(flat, flat, AF.Ln, bias=_f(EPS))          # ln(|PL|+eps)
                S.activation(PL[:, :, 0:2, :], PL[:, :, 0:2, :], AF.Exp,
                             scale=_f(cc["c1"]))                       # u,v
                E.tensor_tensor(PL[:, :, 0, :], PL[:, :, 0, :], PL[:, :, 1, :], OP.add)
                S.activation(PL[:, :, 0, :], PL[:, :, 0, :], AF.Ln)
                S.activation(PL[:, :, 0, :], PL[:, :, 0, :], AF.Exp, scale=_f(cc["c2"]))
                S.activation(PL[:, :, 2, :], PL[:, :, 2, :], AF.Exp, scale=_f(cc["c3"]))
                E.tensor_tensor(PL[:, :, 0, :], PL[:, :, 0, :], PL[:, :, 2, :], OP.add)
                S.activation(PL[:, :, 0, :], PL[:, :, 0, :], AF.Ln)
                S.activation(FF[k][:, :, :].rearrange("p x s -> p s x"),
             PL[:, :, 0, :], AF.Exp, scale=_f(cc["e1"]))

        tc.no_sync_barrier()

        # ---------------- phase 2: occupancy/visibility (exp set) -----------
        # s-innermost layout [P, X, 11]: the occupancy cumsum and the
        # beta-weighted visibility sum each become ONE gated tensor_tensor_scan
        # (state = gate*state + data; gate=0 at s=0 resets per pixel).
        with tc.tile_pool(name="p2", bufs=6) as pool:
            for k in live:
                cc = consts[k]
                E = V
                lr0, nr, j0, nj = rects[k]
                X = nr * nj
                NSS = NS + 1
                g2 = gate[:, 0:X, :].rearrange("p a b -> p (a b)")
                th = pool.tile([P, X, NSS], F32, tag="th", bufs=6)
                # occ = 0.5 + 0.5*tanh(500*(1-F)) == sigmoid(1000*(1-F))
                S.activation(th[:, :, :], FF[k][:, :, :], AF.Tanh,
                             bias=SHARP / 2.0, scale=-SHARP / 2.0)
                E.tensor_scalar(th[:, :, :], th[:, :, :], 0.5, 0.5, OP.mult, OP.add)
                E.tensor_scalar(th[:, :, 0], th[:, :, 0], _f(cc["occ0"]), None, OP.add)

                cum = pool.tile([P, X, NSS], F32, tag="cum", bufs=6)
                E.tensor_tensor_scan(cum[:, :, :].rearrange("p a b -> p (a b)"),
                                     g2, th[:, :, :].rearrange("p a b -> p (a b)"),
                                     0.0, OP.mult, OP.add)
                S.activation(cum[:, :, :], cum[:, :, :], AF.Exp, scale=-TAU)  # vis

                wv = pool.tile([P, X, NSS], F32, tag="wv")
                E.tensor_tensor(wv[:, :, :], cum[:, :, :], betaT[:, 0:X, :], OP.mult)
                E.tensor_tensor_scan(wv[:, :, :].rearrange("p a b -> p (a b)"),
                                     g2, wv[:, :, :].rearrange("p a b -> p (a b)"),
                                     0.0, OP.mult, OP.add)

                acc = pool.tile([P, X], F32, tag="acc")
                E.tensor_tensor(acc[:], wv[:, :, NS], HG[k][:], OP.mult)

                b1 = pool.tile([P, X], F32, tag="b1")
                E.tensor_scalar(b1[:], cum[:, :, 0], 0.5, _f(0.5 * cc["vis0"]),
                                OP.mult, OP.add)
                E.tensor_tensor(b1[:], b1[:], DTT[k][:, 0, :], OP.mult)
                E.tensor_tensor(acc[:], acc[:], b1[:], OP.add)

                b2 = pool.tile([P, X], F32, tag="b2")
                E.tensor_tensor(b2[:], cum[:, :, NS - 1], cum[:, :, NS], OP.add)
                E.scalar_tensor_tensor(b2[:], b2[:], 0.5, DTT[k][:, 1, :], OP.mult, OP.mult)
                E.tensor_tensor(acc[:], acc[:], b2[:], OP.add)

                # dmin[rect] = min(dmin[rect], mask ? depth : FAR)
                E.tensor_scalar(acc[:], acc[:], 1.0, -FAR, OP.mult, OP.add)
                E.tensor_tensor(acc[:], acc[:], MK[k][:], OP.mult)
                E.tensor_scalar(acc[:], acc[:], FAR, None, OP.add)
                dv = dmin[:, lr0:lr0 + nr, j0:j0 + nj]
                V.tensor_tensor(dv, dv, acc[:].rearrange("p (a b) -> p a b", b=nj),
                                OP.min)

        nc.sync.dma_start(out_dram.ap(), dmin[:, :, :])

    # Pre-place the two ACT table loads (natural_log_exp for phase 1,
    # exp_and_others for phase 2/tanh) so bacc's fixpoint inserts none.
    # (CoreSim can't handle the hand-inserted loads; act_loads=False skips.)
    if not act_loads:
        nc.compile()
        return nc
    from concourse.hw_specs import get_activation_tables
    names = list(get_activation_tables(nc.m.arch).keys())
    id_nle = names.index("natural_log_exp_and_others")
    id_exp = names.index("exp_and_others")

    def make_load(set_id):
        ins = mybir.InstLoadActFuncSet(
            name=nc.get_next_instruction_name(), act_func_set_id=set_id,
            ins=[], outs=[])
        ins.engine = nc.scalar.engine
        return ins

    for blk in nc.main_func.blocks:
        il = blk.instructions
        first_act = next((i for i, x in enumerate(il)
                          if isinstance(x, mybir.InstActivation)), None)
        if first_act is None:
            continue
        first_tanh = next((i for i, x in enumerate(il)
                           if isinstance(x, mybir.InstActivation)
                           and x.func == AF.Tanh), None)
        il.insert(first_act, make_load(id_nle))
        if first_tanh is not None:
            il.insert(first_tanh + 1, make_load(id_exp))

    nc.compile()
    return nc


def _shard_rays(rays_d):
    """-> per-core arrays [128, 3, 45, 5]; core c owns rows 8*lr+c."""
    rd = np.asarray(rays_d, np.float32)
    out = []
    for c in range(N_CORES):
        sub = rd[c::N_CORES]                         # (45, 640, 3)
        arr = sub.reshape(NRL, NJ, 128, 3).transpose(2, 3, 0, 1)
        out.append(np.ascontiguousarray(arr))        # (128, 3, 45, 5)
    return out


def _unshard(outs):
    """outs: list of 8 arrays [128, 45, 5] -> (360, 640)."""
    full = np.empty((HS, WS), np.float32)
    for c in range(N_CORES):
        full[c::N_CORES] = outs[c].transpose(1, 2, 0).reshape(NRL, WS)
    return full


def kernel(sq_poses, sq_params, rays_d, rays_o, t, **run_kwargs):
    consts, tv, beta = _host_consts(sq_poses, sq_params, rays_o, t)
    rects = _host_rects(consts, rays_d)
    nc = build_program(consts, tv, beta, rects)
    planes = _shard_rays(rays_d)
    in_maps = [{"rdin": planes[c]} for c in range(N_CORES)]
    res = run_bass_kernel_spmd(nc, in_maps, core_ids=list(range(N_CORES)), **run_kwargs)
    outs = [res.results[c]["depth"] for c in range(N_CORES)]
    out = _unshard(outs).astype(np.float32)
    kernel.last_result = res
    return out


kernel.last_result = None


# revision 24
# speedup vs baseline: 1.0418x; 1.0405x over previous
"""Trainium2 Bass kernel for nn_DepthRenderer (superquadric depth renderer).

Sharding: rows round-robin over 8 cores (core c owns image rows r = 8*lr+c,
lr=0..44).  Per-core layout [128 lanes, 45 lrows, 5 xblocks]; lane = x%128,
xblock = x//128.  Each core renders all 8 SQs (constants baked as immediates
into one SPMD program) and min-accumulates depth on device; host concatenates.

Sparsity: a SQ can only influence pixels where the ray enters its bounding
sphere: h(d) = (b.d)^2 - (C-3) * d^T A d > 0 (homogeneous quadratic in the ray
direction, so normalization-free).  The host evaluates h on a coarse pixel
subgrid, takes the bounding rectangle (+margin, rows rounded to multiples of 8
so the rect is the SAME static view on every core), and the device program
processes only that rect per SQ (~4.7x less work).  Pixels outside the rect
keep depth FAR; rect pixels use the exact in-rect mask, and the premask
boundary is depth-continuous (grazing rays integrate to ~FAR), so the coarse
rect is safe.

Math notes (exact rewrites of the reference, up to fp rounding):
  - a == sizes  =>  X = |loc|/a + eps = |pts_loc| + eps  (sizes cancel)
  - ||td * sizes|| = ||d|| * rinv  (rotation invariance)
  - dt0  = ||pts_loc[0]*s + R^T p||,  dt10 = ||(PL10-PL9)*s||  with
    PL10 = loc_far/s = 1.5*u - (R^T p)/s
  - sqrt(x) = exp(0.5*ln(x)); sigmoid(x) = 0.5 + 0.5*tanh(x/2)
  - phase 1 (pow chains) uses the natural_log_exp ACT table set, phase 2
    (tanh occupancy + visibility exp) uses exp_and_others; both loads are
    pre-placed so bacc inserts no further table switches.
"""

from contextlib import ExitStack

import numpy as np

import concourse.bass as bass
import concourse.bacc as bacc
import concourse.mybir as mybir
from concourse import tile
from bass_rust import add_dep_helper
from concourse.bass_utils import run_bass_kernel_spmd

F32 = mybir.dt.float32
AF = mybir.ActivationFunctionType
OP = mybir.AluOpType

# renderer constants (match the nn.Module init)
HS, WS = 360, 640
NEAR, FAR = 0.0, 1.5
NS = 10
SHARP = 1000.0
TAU = 100.0
N_SQ = 8
EPS = 1e-6

N_CORES = 8
NRL = HS // N_CORES       # 45 local rows per core
NJ = WS // 128            # 5 x-blocks
NCOL = NRL * NJ           # 225 columns per core
P = 128


def _f(x):
    return float(np.float32(x))


def _host_consts(sq_poses, sq_params, rays_o, t):
    """Per-SQ scalars, computed in float64 from the f32 inputs."""
    sq_poses = np.asarray(sq_poses, np.float64)
    sq_params = np.asarray(sq_params, np.float64)
    rays_o = np.asarray(rays_o, np.float64)
    t = np.asarray(t, np.float64)

    consts = []
    for k in range(N_SQ):
        R = sq_poses[k, :3, :3]
        p = sq_poses[k, :3, 3]
        s = sq_params[k, 0:3]
        e1 = sq_params[k, 3]
        e2 = sq_params[k, 4]

        M1 = R.T / s[:, None]            # u = M1 @ d = (R^T d)/s
        tc = (R.T @ (rays_o - p)) / s
        rp = R.T @ p                      # loc(near) = -rp
        rps = rp / s
        c1 = 2.0 / e2
        c2 = e2 / e1
        c3 = 2.0 / e1

        # near-point occupancy (constant per SQ)
        Xn = np.abs(-rp) / s + EPS
        fN = (Xn[0] ** c1 + Xn[1] ** c1) ** c2 + Xn[2] ** c3
        Fn = fN ** e1
        with np.errstate(over="ignore"):
            occ0 = 1.0 / (1.0 + np.exp(-SHARP * (1.0 - Fn)))
        vis0 = np.exp(-TAU * occ0)

        consts.append(dict(
            M1=M1, tc=tc, rp=rp, rps=rps, s=s,
            c1=c1, c2=c2, c3=c3, e1=e1,
            occ0=occ0, vis0=vis0,
        ))

    # segment weights from t (shared across SQs)
    dt_abs = np.abs(np.diff(t))          # |t_i - t_{i-1}|, i=1..9
    beta = np.zeros(11)                  # weight of v_s (s=1..10) in inner sum
    for i in range(1, NS):               # inner gaps i=1..9 use v_i, v_{i+1}
        beta[i] += 0.5 * dt_abs[i - 1]
        beta[i + 1] += 0.5 * dt_abs[i - 1]
    return consts, t, beta


def _host_rects(consts, rays_d):
    """Per-SQ (lr0, nr, j0, nj) bounding rect, identical across cores.

    h(d) = (b.d)^2 - (C-3) d^T A d is degree-2 homogeneous in d, so the
    coarse-subgrid sign test needs no ray normalization.  Conservative by a
    9px margin (>> 3px grid step; min blob diameter is ~40px for any SQ with
    C comfortably > 3).  Rows rounded to multiples of 8 so that every core's
    local-row range is the same [lr0, lr0+nr).
    """
    d = np.asarray(rays_d, np.float64)
    ys = np.arange(0, HS, 3)
    xs = np.arange(0, WS, 3)
    sub = d[np.ix_(ys, xs)]
    rects = []
    for cc in consts:
        M1, tcv = cc["M1"], cc["tc"]
        C = float((tcv ** 2).sum())
        if C <= 3.5:                      # near/inside bounding sphere: dense
            rects.append((0, NRL, 0, NJ))
            continue
        A = M1.T @ M1
        b = M1.T @ tcv
        hq = (sub @ b) ** 2 - (C - 3.0) * np.einsum("yxi,ij,yxj->yx", sub, A, sub)
        hit = hq > 0
        if not hit.any():
            rects.append(None)
            continue
        ryy, rxx = np.where(hit)
        r0 = max(0, int(ys[ryy.min()]) - 9)
        r1 = min(HS - 1, int(ys[ryy.max()]) + 9)
        x0 = max(0, int(xs[rxx.min()]) - 9)
        x1 = min(WS - 1, int(xs[rxx.max()]) + 9)
        r0 = (r0 // 8) * 8
        r1 = min(HS, ((r1 + 8) // 8) * 8) - 1
        lr0, nr = r0 // 8, (r1 - r0 + 1) // 8
        j0, j1 = x0 // 128, x1 // 128
        rects.append((lr0, nr, j0, j1 - j0 + 1))
    return rects


def build_program(consts, t, beta, rects, act_loads=True):
    """One SPMD program; input rdin [128,3,45,5], output depth [128,45,5]."""
    nc = bacc.Bacc("TRN2", target_bir_lowering=False, debug=False,
                   enable_asserts=False, num_devices=N_CORES)

    rd_dram = nc.dram_tensor("rdin", [P, 3, NRL, NJ], F32, kind="ExternalInput")
    out_dram = nc.dram_tensor("depth", [P, NRL, NJ], F32, kind="ExternalOutput")

    # const APs for activation biases (only 0.0/1.0 are pre-registered)
    def reg_const(v):
        v = _f(v)
        if (F32, v) not in nc.const_aps.aps:
            th = nc.alloc_sbuf_tensor(f"constap{len(nc.const_aps.aps)}", [128, 1], F32)
            nc.gpsimd.memset(th.ap(), v)
            nc.const_aps.aps[(F32, v)] = th.ap()

    reg_const(EPS)
    reg_const(SHARP / 2.0)
    for cc in consts:
        for j in range(3):
            reg_const(cc["rp"][j])
    nc.all_engine_barrier()

    live = [k for k in range(N_SQ) if rects[k] is not None]

    with tile.TileContext(nc) as tc, ExitStack() as es:
        V = nc.vector
        S = nc.scalar
        persist = es.enter_context(tc.tile_pool(name="persist", bufs=1))

        # ---- shared loads & per-core shared prep ----
        rd = persist.tile([P, 3, NRL, NJ], F32, name="rd")
        nc.sync.dma_start(rd[:, :, :, :], rd_dram.ap())

        rdsq = persist.tile([P, 3, NRL, NJ], F32, name="rdsq")
        S.activation(rdsq[:, :, :, :], rd[:, :, :, :], AF.Square)
        nd2 = persist.tile([P, NRL, NJ], F32, name="nd2")
        V.tensor_tensor(nd2[:, :, :], rdsq[:, 0, :, :], rdsq[:, 1, :, :], OP.add)
        V.tensor_tensor(nd2[:, :, :], nd2[:, :, :], rdsq[:, 2, :, :], OP.add)
        nd = persist.tile([P, NRL, NJ], F32, name="nd")
        S.activation(nd[:, :, :], nd2[:, :, :], AF.Ln)
        S.activation(nd[:, :, :], nd[:, :, :], AF.Exp, scale=0.5)

        dmin = persist.tile([P, NRL, NJ], F32, name="dmin")
        V.memset(dmin[:, :, :], FAR)

        # persistent per-SQ results for phase 2 (sized per rect)
        FF, MK, HG, DTT = {}, {}, {}, {}
        for k in live:
            lr0, nr, j0, nj = rects[k]
            X = nr * nj
            FF[k] = persist.tile([P, NS + 1, X], F32, name=f"FF{k}")
            MK[k] = persist.tile([P, X], F32, name=f"MK{k}")
            HG[k] = persist.tile([P, X], F32, name=f"HG{k}")
            DTT[k] = persist.tile([P, 2, X], F32, name=f"DTT_{k}")

        # ---------------- phase 1: per-SQ F chains (ln/exp table set) -------
        with tc.tile_pool(name="p1", bufs=3) as pool:
            for k in live:
                cc = consts[k]
                E = V
                lr0, nr, j0, nj = rects[k]
                X = nr * nj
                M1, tcv, rp, rps, s = cc["M1"], cc["tc"], cc["rp"], cc["rps"], cc["s"]

                def r4(ap2):   # [P, X] compact view -> [P, nr, nj]
                    return ap2.rearrange("p (a b) -> p a b", b=nj)

                # compact copies of the rect slice of rd (and ||d||)
                rdc = pool.tile([P, 3, X], F32, tag="rdc")
                E.tensor_copy(rdc[:, :, :].rearrange("p c (a b) -> p c a b", b=nj),
                              rd[:, :, lr0:lr0 + nr, j0:j0 + nj])
                ndc = pool.tile([P, X], F32, tag="ndc")
                E.tensor_copy(r4(ndc[:, :]), nd[:, lr0:lr0 + nr, j0:j0 + nj])

                u = pool.tile([P, 3, X], F32, tag="u")
                for j in range(3):
                    E.tensor_scalar(u[:, j, :], rdc[:, 0, :], _f(M1[j, 0]), None, OP.mult)
                    E.scalar_tensor_tensor(u[:, j, :], rdc[:, 1, :], _f(M1[j, 1]), u[:, j, :], OP.mult, OP.add)
                    E.scalar_tensor_tensor(u[:, j, :], rdc[:, 2, :], _f(M1[j, 2]), u[:, j, :], OP.mult, OP.add)

                usq = pool.tile([P, 3, X], F32, tag="usq")
                E.tensor_tensor(usq[:, :, :], u[:, :, :], u[:, :, :], OP.mult)
                nu2 = pool.tile([P, X], F32, tag="nu2")
                E.tensor_tensor(nu2[:], usq[:, 0, :], usq[:, 1, :], OP.add)
                E.tensor_tensor(nu2[:], nu2[:], usq[:, 2, :], OP.add)

                rinv = pool.tile([P, X], F32, tag="rinv")
                S.activation(rinv[:], nu2[:], AF.Ln)
                S.activation(rinv[:], rinv[:], AF.Exp, scale=-0.5)

                td = pool.tile([P, 3, X], F32, tag="td")
                for j in range(3):
                    E.tensor_tensor(td[:, j, :], u[:, j, :], rinv[:], OP.mult)

                d1 = pool.tile([P, X], F32, tag="d1")
                E.tensor_scalar(d1[:], td[:, 0, :], _f(tcv[0]), None, OP.mult)
                E.scalar_tensor_tensor(d1[:], td[:, 1, :], _f(tcv[1]), d1[:], OP.mult, OP.add)
                E.scalar_tensor_tensor(d1[:], td[:, 2, :], _f(tcv[2]), d1[:], OP.mult, OP.add)
                proj = pool.tile([P, X], F32, tag="proj")
                S.activation(proj[:], d1[:], AF.Abs)

                cen = pool.tile([P, 3, X], F32, tag="cen")
                for j in range(3):
                    E.tensor_tensor(cen[:, j, :], proj[:], td[:, j, :], OP.mult)
                    E.tensor_scalar(cen[:, j, :], cen[:, j, :], _f(tcv[j]), None, OP.add)

                csq = pool.tile([P, 3, X], F32, tag="usq")
                E.tensor_tensor(csq[:, :, :], cen[:, :, :], cen[:, :, :], OP.mult)
                m3 = pool.tile([P, X], F32, tag="m3")
                E.tensor_tensor(m3[:], csq[:, 0, :], csq[:, 1, :], OP.add)
                E.tensor_tensor(m3[:], m3[:], csq[:, 2, :], OP.add)
                # m3 = 3 - dist^2 ; mask = m3 > 0 ; hclsq = max(m3, 1e-12)
                E.tensor_scalar(m3[:], m3[:], -1.0, 3.0, OP.mult, OP.add)
                E.tensor_scalar(MK[k][:], m3[:], 0.0, None, OP.is_gt)
                E.tensor_scalar(m3[:], m3[:], 1e-12, None, OP.max)

                hcl = pool.tile([P, X], F32, tag="hcl")
                S.activation(hcl[:], m3[:], AF.Ln)
                S.activation(hcl[:], hcl[:], AF.Exp, scale=0.5)

                # hg = hcl * ||d|| * rinv
                E.tensor_tensor(HG[k][:], ndc[:], rinv[:], OP.mult)
                E.tensor_tensor(HG[k][:], HG[k][:], hcl[:], OP.mult)

                htd = pool.tile([P, 3, X], F32, tag="htd")
                for j in range(3):
                    E.tensor_tensor(htd[:, j, :], hcl[:], td[:, j, :], OP.mult)

                # PL slots 0..9: cen + t_s*htd ; slot 10: 1.5*u - rp/s
                PL = pool.tile([P, NS + 1, 3, X], F32, tag="PL", bufs=3)
                for si in range(NS):
                    E.scalar_tensor_tensor(PL[:, si, :, :], htd[:, :, :], _f(t[si]),
                                           cen[:, :, :], OP.mult, OP.add)
                for j in range(3):
                    E.tensor_scalar(PL[:, NS, j, :], u[:, j, :], 1.5, _f(-rps[j]),
                                    OP.mult, OP.add)

                # dt0 = ||PL0*s + rp|| ; dt10 = ||(PL10-PL9)*s||
                q3 = pool.tile([P, 3, X], F32, tag="q3")
                for j in range(3):
                    S.activation(q3[:, j, :], PL[:, 0, j, :], AF.Square,
                                 bias=_f(rp[j]), scale=_f(s[j]))
                dtt = DTT[k]
                E.tensor_tensor(dtt[:, 0, :], q3[:, 0, :], q3[:, 1, :], OP.add)
                E.tensor_tensor(dtt[:, 0, :], dtt[:, 0, :], q3[:, 2, :], OP.add)

                df = pool.tile([P, 3, X], F32, tag="q3b")
                E.tensor_tensor(df[:, :, :], PL[:, NS, :, :], PL[:, NS - 1, :, :], OP.subtract)
                for j in range(3):
                    S.activation(df[:, j, :], df[:, j, :], AF.Square, scale=_f(s[j]))
                E.tensor_tensor(dtt[:, 1, :], df[:, 0, :], df[:, 1, :], OP.add)
                E.tensor_tensor(dtt[:, 1, :], dtt[:, 1, :], df[:, 2, :], OP.add)
                S.activation(dtt[:, :, :], dtt[:, :, :], AF.Ln)
                S.activation(dtt[:, :, :], dtt[:, :, :], AF.Exp, scale=0.5)

                # F chain, in place over PL
                flat = PL[:, :, :, :]
                S.activation(flat, flat, AF.Abs)                       # |PL|
                S.activation(flat, flat, AF.Ln, bias=_f(EPS))          # ln(|PL|+eps)
                S.activation(PL[:, :, 0:2, :], PL[:, :, 0:2, :], AF.Exp,
                             scale=_f(cc["c1"]))                       # u,v
                E.tensor_tensor(PL[:, :, 0, :], PL[:, :, 0, :], PL[:, :, 1, :], OP.add)
                S.activation(PL[:, :, 0, :], PL[:, :, 0, :], AF.Ln)
                S.activation(PL[:, :, 0, :], PL[:, :, 0, :], AF.Exp, scale=_f(cc["c2"]))
                S.activation(PL[:, :, 2, :], PL[:, :, 2, :], AF.Exp, scale=_f(cc["c3"]))
                E.tensor_tensor(PL[:, :, 0, :], PL[:, :, 0, :], PL[:, :, 2, :], OP.add)
                S.activation(PL[:, :, 0, :], PL[:, :, 0, :], AF.Ln)
                last_p1_act = S.activation(FF[k][:, :, :], PL[:, :, 0, :], AF.Exp, scale=_f(cc["e1"]))


        # ---------------- phase 2: occupancy/visibility (exp set) -----------
        with tc.tile_pool(name="p2", bufs=3) as pool:
            for k in live:
                cc = consts[k]
                E = V
                lr0, nr, j0, nj = rects[k]
                X = nr * nj
                th = pool.tile([P, NS + 1, X], F32, tag="th", bufs=3)
                # occ = 0.5 + 0.5*tanh(500*(1-F)) == sigmoid(1000*(1-F))
                th_i = S.activation(th[:, :, :], FF[k][:, :, :], AF.Tanh,
                                    bias=SHARP / 2.0, scale=-SHARP / 2.0)
                add_dep_helper(th_i.ins, last_p1_act.ins, sync=False,
                               reason="ACT table phase order")
                E.tensor_scalar(th[:, :, :], th[:, :, :], 0.5, 0.5, OP.mult, OP.add)

                cum = pool.tile([P, NS + 1, X], F32, tag="cum", bufs=3)
                E.tensor_scalar(cum[:, 0, :], th[:, 0, :], _f(cc["occ0"]), None, OP.add)
                for si in range(1, NS + 1):
                    E.tensor_tensor(cum[:, si, :], cum[:, si - 1, :], th[:, si, :], OP.add)
                vis_i = S.activation(cum[:, :, :], cum[:, :, :], AF.Exp, scale=-TAU)
                add_dep_helper(vis_i.ins, last_p1_act.ins, sync=False,
                               reason="ACT table phase order")

                acc = pool.tile([P, X], F32, tag="acc")
                E.tensor_scalar(acc[:], cum[:, 0, :], _f(beta[1]), None, OP.mult)
                for si in range(1, NS):
                    E.scalar_tensor_tensor(acc[:], cum[:, si, :], _f(beta[si + 1]),
                                           acc[:], OP.mult, OP.add)
                E.tensor_tensor(acc[:], acc[:], HG[k][:], OP.mult)

                b1 = pool.tile([P, X], F32, tag="b1")
                E.tensor_scalar(b1[:], cum[:, 0, :], 0.5, _f(0.5 * cc["vis0"]),
                                OP.mult, OP.add)
                E.tensor_tensor(b1[:], b1[:], DTT[k][:, 0, :], OP.mult)
                E.tensor_tensor(acc[:], acc[:], b1[:], OP.add)

                b2 = pool.tile([P, X], F32, tag="b2")
                E.tensor_tensor(b2[:], cum[:, NS - 1, :], cum[:, NS, :], OP.add)
                E.scalar_tensor_tensor(b2[:], b2[:], 0.5, DTT[k][:, 1, :], OP.mult, OP.mult)
                E.tensor_tensor(acc[:], acc[:], b2[:], OP.add)

                # dmin[rect] = min(dmin[rect], mask ? depth : FAR)
                E.tensor_scalar(acc[:], acc[:], 1.0, -FAR, OP.mult, OP.add)
                E.tensor_tensor(acc[:], acc[:], MK[k][:], OP.mult)
                E.tensor_scalar(acc[:], acc[:], FAR, None, OP.add)
                dv = dmin[:, lr0:lr0 + nr, j0:j0 + nj]
                V.tensor_tensor(dv, dv, acc[:].rearrange("p (a b) -> p a b", b=nj),
                                OP.min)

        nc.sync.dma_start(out_dram.ap(), dmin[:, :, :])

    # Pre-place the two ACT table loads (natural_log_exp for phase 1,
    # exp_and_others for phase 2/tanh) so bacc's fixpoint inserts none.
    # (CoreSim can't handle the hand-inserted loads; act_loads=False skips.)
    if not act_loads:
        nc.compile()
        return nc
    from concourse.hw_specs import get_activation_tables
    names = list(get_activation_tables(nc.m.arch).keys())
    id_nle = names.index("natural_log_exp_and_others")
    id_exp = names.index("exp_and_others")

    def make_load(set_id):
        ins = mybir.InstLoadActFuncSet(
            name=nc.get_next_instruction_name(), act_func_set_id=set_id,
            ins=[], outs=[])
        ins.engine = nc.scalar.engine
        return ins

    for blk in nc.main_func.blocks:
        il = blk.instructions
        first_act = next((i for i, x in enumerate(il)
                          if isinstance(x, mybir.InstActivation)), None)
        if first_act is None:
            continue
        first_tanh = next((i for i, x in enumerate(il)
                           if isinstance(x, mybir.InstActivation)
                           and x.func == AF.Tanh), None)
        il.insert(first_act, make_load(id_nle))
        if first_tanh is not None:
            il.insert(first_tanh + 1, make_load(id_exp))

    nc.compile()
    return nc


def _shard_rays(rays_d):
    """-> per-core arrays [128, 3, 45, 5]; core c owns rows 8*lr+c."""
    rd = np.asarray(rays_d, np.float32)
    out = []
    for c in range(N_CORES):
        sub = rd[c::N_CORES]                         # (45, 640, 3)
        arr = sub.reshape(NRL, NJ, 128, 3).transpose(2, 3, 0, 1)
        out.append(np.ascontiguousarray(arr))        # (128, 3, 45, 5)
    return out


def _unshard(outs):
    """outs: list of 8 arrays [128, 45, 5] -> (360, 640)."""
    full = np.empty((HS, WS), np.float32)
    for c in range(N_CORES):
        full[c::N_CORES] = outs[c].transpose(1, 2, 0).reshape(NRL, WS)
    return full


def kernel(sq_poses, sq_params, rays_d, rays_o, t, **run_kwargs):
    consts, tv, beta = _host_consts(sq_poses, sq_params, rays_o, t)
    rects = _host_rects(consts, rays_d)
    nc = build_program(consts, tv, beta, rects)
    planes = _shard_rays(rays_d)
    in_maps = [{"rdin": planes[c]} for c in range(N_CORES)]
    res = run_bass_kernel_spmd(nc, in_maps, core_ids=list(range(N_CORES)), **run_kwargs)
    outs = [res.results[c]["depth"] for c in range(N_CORES)]
    out = _unshard(outs).astype(np.float32)
    kernel.last_result = res
    return out


kernel.last_result = None


# revision 25
# speedup vs baseline: 1.0422x; 1.0004x over previous
"""Trainium2 Bass kernel for nn_DepthRenderer (superquadric depth renderer).

Sharding: rows round-robin over 8 cores (core c owns image rows r = 8*lr+c,
lr=0..44).  Per-core layout [128 lanes, 45 lrows, 5 xblocks]; lane = x%128,
xblock = x//128.  Each core renders all 8 SQs (constants baked as immediates
into one SPMD program) and min-accumulates depth on device; host concatenates.

Sparsity: a SQ can only influence pixels where the ray enters its bounding
sphere: h(d) = (b.d)^2 - (C-3) * d^T A d > 0 (homogeneous quadratic in the ray
direction, so normalization-free).  The host evaluates h on a coarse pixel
subgrid, takes the bounding rectangle (+margin, rows rounded to multiples of 8
so the rect is the SAME static view on every core), and the device program
processes only that rect per SQ (~4.7x less work).  Pixels outside the rect
keep depth FAR; rect pixels use the exact in-rect mask, and the premask
boundary is depth-continuous (grazing rays integrate to ~FAR), so the coarse
rect is safe.

Math notes (exact rewrites of the reference, up to fp rounding):
  - a == sizes  =>  X = |loc|/a + eps = |pts_loc| + eps  (sizes cancel)
  - ||td * sizes|| = ||d|| * rinv  (rotation invariance)
  - dt0  = ||pts_loc[0]*s + R^T p||,  dt10 = ||(PL10-PL9)*s||  with
    PL10 = loc_far/s = 1.5*u - (R^T p)/s
  - sqrt(x) = exp(0.5*ln(x)); sigmoid(x) = 0.5 + 0.5*tanh(x/2)
  - phase 1 (pow chains) uses the natural_log_exp ACT table set, phase 2
    (tanh occupancy + visibility exp) uses exp_and_others; both loads are
    pre-placed so bacc inserts no further table switches.
"""

from contextlib import ExitStack

import numpy as np

import concourse.bass as bass
import concourse.bacc as bacc
import concourse.mybir as mybir
from concourse import tile
from concourse.bass_utils import run_bass_kernel_spmd

F32 = mybir.dt.float32
AF = mybir.ActivationFunctionType
OP = mybir.AluOpType

# renderer constants (match the nn.Module init)
HS, WS = 360, 640
NEAR, FAR = 0.0, 1.5
NS = 10
SHARP = 1000.0
TAU = 100.0
N_SQ = 8
EPS = 1e-6

N_CORES = 8
NRL = HS // N_CORES       # 45 local rows per core
NJ = WS // 128            # 5 x-blocks
NCOL = NRL * NJ           # 225 columns per core
P = 128


def _f(x):
    return float(np.float32(x))


def _host_consts(sq_poses, sq_params, rays_o, t):
    """Per-SQ scalars, computed in float64 from the f32 inputs."""
    sq_poses = np.asarray(sq_poses, np.float64)
    sq_params = np.asarray(sq_params, np.float64)
    rays_o = np.asarray(rays_o, np.float64)
    t = np.asarray(t, np.float64)

    consts = []
    for k in range(N_SQ):
        R = sq_poses[k, :3, :3]
        p = sq_poses[k, :3, 3]
        s = sq_params[k, 0:3]
        e1 = sq_params[k, 3]
        e2 = sq_params[k, 4]

        M1 = R.T / s[:, None]            # u = M1 @ d = (R^T d)/s
        tc = (R.T @ (rays_o - p)) / s
        rp = R.T @ p                      # loc(near) = -rp
        rps = rp / s
        c1 = 2.0 / e2
        c2 = e2 / e1
        c3 = 2.0 / e1

        # near-point occupancy (constant per SQ)
        Xn = np.abs(-rp) / s + EPS
        fN = (Xn[0] ** c1 + Xn[1] ** c1) ** c2 + Xn[2] ** c3
        Fn = fN ** e1
        with np.errstate(over="ignore"):
            occ0 = 1.0 / (1.0 + np.exp(-SHARP * (1.0 - Fn)))
        vis0 = np.exp(-TAU * occ0)

        consts.append(dict(
            M1=M1, tc=tc, rp=rp, rps=rps, s=s,
            c1=c1, c2=c2, c3=c3, e1=e1,
            occ0=occ0, vis0=vis0,
        ))

    # segment weights from t (shared across SQs)
    dt_abs = np.abs(np.diff(t))          # |t_i - t_{i-1}|, i=1..9
    beta = np.zeros(11)                  # weight of v_s (s=1..10) in inner sum
    for i in range(1, NS):               # inner gaps i=1..9 use v_i, v_{i+1}
        beta[i] += 0.5 * dt_abs[i - 1]
        beta[i + 1] += 0.5 * dt_abs[i - 1]
    return consts, t, beta


def _host_rects(consts, rays_d):
    """Per-SQ (lr0, nr, j0, nj) bounding rect, identical across cores.

    h(d) = (b.d)^2 - (C-3) d^T A d is degree-2 homogeneous in d, so the
    coarse-subgrid sign test needs no ray normalization.  Conservative by a
    9px margin (>> 3px grid step; min blob diameter is ~40px for any SQ with
    C comfortably > 3).  Rows rounded to multiples of 8 so that every core's
    local-row range is the same [lr0, lr0+nr).
    """
    d = np.asarray(rays_d, np.float64)
    ys = np.arange(0, HS, 3)
    xs = np.arange(0, WS, 3)
    sub = d[np.ix_(ys, xs)]
    rects = []
    for cc in consts:
        M1, tcv = cc["M1"], cc["tc"]
        C = float((tcv ** 2).sum())
        if C <= 3.5:                      # near/inside bounding sphere: dense
            rects.append((0, NRL, 0, NJ))
            continue
        A = M1.T @ M1
        b = M1.T @ tcv
        hq = (sub @ b) ** 2 - (C - 3.0) * np.einsum("yxi,ij,yxj->yx", sub, A, sub)
        hit = hq > 0
        if not hit.any():
            rects.append(None)
            continue
        ryy, rxx = np.where(hit)
        r0 = max(0, int(ys[ryy.min()]) - 9)
        r1 = min(HS - 1, int(ys[ryy.max()]) + 9)
        x0 = max(0, int(xs[rxx.min()]) - 9)
        x1 = min(WS - 1, int(xs[rxx.max()]) + 9)
        r0 = (r0 // 8) * 8
        r1 = min(HS, ((r1 + 8) // 8) * 8) - 1
        lr0, nr = r0 // 8, (r1 - r0 + 1) // 8
        j0, j1 = x0 // 128, x1 // 128
        rects.append((lr0, nr, j0, j1 - j0 + 1))
    return rects


def build_program(consts, t, beta, rects, act_loads=True):
    """One SPMD program; input rdin [128,3,45,5], output depth [128,45,5]."""
    nc = bacc.Bacc("TRN2", target_bir_lowering=False, debug=False,
                   enable_asserts=False, num_devices=N_CORES)

    rd_dram = nc.dram_tensor("rdin", [P, 3, NRL, NJ], F32, kind="ExternalInput")
    out_dram = nc.dram_tensor("depth", [P, NRL, NJ], F32, kind="ExternalOutput")

    # const APs for activation biases (only 0.0/1.0 are pre-registered)
    def reg_const(v):
        v = _f(v)
        if (F32, v) not in nc.const_aps.aps:
            th = nc.alloc_sbuf_tensor(f"constap{len(nc.const_aps.aps)}", [128, 1], F32)
            nc.gpsimd.memset(th.ap(), v)
            nc.const_aps.aps[(F32, v)] = th.ap()

    reg_const(EPS)
    reg_const(SHARP / 2.0)
    for cc in consts:
        for j in range(3):
            reg_const(cc["rp"][j])
    nc.all_engine_barrier()

    live = [k for k in range(N_SQ) if rects[k] is not None]

    with tile.TileContext(nc) as tc, ExitStack() as es:
        V = nc.vector
        S = nc.scalar
        persist = es.enter_context(tc.tile_pool(name="persist", bufs=1))

        # ---- shared loads & per-core shared prep ----
        rd = persist.tile([P, 3, NRL, NJ], F32, name="rd")
        nc.sync.dma_start(rd[:, :, :, :], rd_dram.ap())

        rdsq = persist.tile([P, 3, NRL, NJ], F32, name="rdsq")
        S.activation(rdsq[:, :, :, :], rd[:, :, :, :], AF.Square)
        nd2 = persist.tile([P, NRL, NJ], F32, name="nd2")
        V.tensor_tensor(nd2[:, :, :], rdsq[:, 0, :, :], rdsq[:, 1, :, :], OP.add)
        V.tensor_tensor(nd2[:, :, :], nd2[:, :, :], rdsq[:, 2, :, :], OP.add)
        nd = persist.tile([P, NRL, NJ], F32, name="nd")
        S.activation(nd[:, :, :], nd2[:, :, :], AF.Ln)
        S.activation(nd[:, :, :], nd[:, :, :], AF.Exp, scale=0.5)

        dmin = persist.tile([P, NRL, NJ], F32, name="dmin")
        V.memset(dmin[:, :, :], FAR)

        # persistent per-SQ results for phase 2 (sized per rect)
        FF, MK, HG, DTT = {}, {}, {}, {}
        for k in live:
            lr0, nr, j0, nj = rects[k]
            X = nr * nj
            FF[k] = persist.tile([P, NS + 1, X], F32, name=f"FF{k}")
            MK[k] = persist.tile([P, X], F32, name=f"MK{k}")
            HG[k] = persist.tile([P, X], F32, name=f"HG{k}")
            DTT[k] = persist.tile([P, 2, X], F32, name=f"DTT_{k}")

        # ---------------- phase 1: per-SQ F chains (ln/exp table set) -------
        with tc.tile_pool(name="p1", bufs=3) as pool:
            for k in live:
                cc = consts[k]
                E = V
                lr0, nr, j0, nj = rects[k]
                X = nr * nj
                M1, tcv, rp, rps, s = cc["M1"], cc["tc"], cc["rp"], cc["rps"], cc["s"]

                def r4(ap2):   # [P, X] compact view -> [P, nr, nj]
                    return ap2.rearrange("p (a b) -> p a b", b=nj)

                # compact copies of the rect slice of rd (and ||d||)
                rdc = pool.tile([P, 3, X], F32, tag="rdc")
                E.tensor_copy(rdc[:, :, :].rearrange("p c (a b) -> p c a b", b=nj),
                              rd[:, :, lr0:lr0 + nr, j0:j0 + nj])
                ndc = pool.tile([P, X], F32, tag="ndc")
                E.tensor_copy(r4(ndc[:, :]), nd[:, lr0:lr0 + nr, j0:j0 + nj])

                u = pool.tile([P, 3, X], F32, tag="u")
                for j in range(3):
                    E.tensor_scalar(u[:, j, :], rdc[:, 0, :], _f(M1[j, 0]), None, OP.mult)
                    E.scalar_tensor_tensor(u[:, j, :], rdc[:, 1, :], _f(M1[j, 1]), u[:, j, :], OP.mult, OP.add)
                    E.scalar_tensor_tensor(u[:, j, :], rdc[:, 2, :], _f(M1[j, 2]), u[:, j, :], OP.mult, OP.add)

                usq = pool.tile([P, 3, X], F32, tag="usq")
                E.tensor_tensor(usq[:, :, :], u[:, :, :], u[:, :, :], OP.mult)
                nu2 = pool.tile([P, X], F32, tag="nu2")
                E.tensor_tensor(nu2[:], usq[:, 0, :], usq[:, 1, :], OP.add)
                E.tensor_tensor(nu2[:], nu2[:], usq[:, 2, :], OP.add)

                rinv = pool.tile([P, X], F32, tag="rinv")
                S.activation(rinv[:], nu2[:], AF.Ln)
                S.activation(rinv[:], rinv[:], AF.Exp, scale=-0.5)

                td = pool.tile([P, 3, X], F32, tag="td")
                for j in range(3):
                    E.tensor_tensor(td[:, j, :], u[:, j, :], rinv[:], OP.mult)

                d1 = pool.tile([P, X], F32, tag="d1")
                E.tensor_scalar(d1[:], td[:, 0, :], _f(tcv[0]), None, OP.mult)
                E.scalar_tensor_tensor(d1[:], td[:, 1, :], _f(tcv[1]), d1[:], OP.mult, OP.add)
                E.scalar_tensor_tensor(d1[:], td[:, 2, :], _f(tcv[2]), d1[:], OP.mult, OP.add)
                proj = pool.tile([P, X], F32, tag="proj")
                S.activation(proj[:], d1[:], AF.Abs)

                cen = pool.tile([P, 3, X], F32, tag="cen")
                for j in range(3):
                    E.tensor_tensor(cen[:, j, :], proj[:], td[:, j, :], OP.mult)
                    E.tensor_scalar(cen[:, j, :], cen[:, j, :], _f(tcv[j]), None, OP.add)

                csq = pool.tile([P, 3, X], F32, tag="usq")
                E.tensor_tensor(csq[:, :, :], cen[:, :, :], cen[:, :, :], OP.mult)
                m3 = pool.tile([P, X], F32, tag="m3")
                E.tensor_tensor(m3[:], csq[:, 0, :], csq[:, 1, :], OP.add)
                E.tensor_tensor(m3[:], m3[:], csq[:, 2, :], OP.add)
                # m3 = 3 - dist^2 ; mask = m3 > 0 ; hclsq = max(m3, 1e-12)
                E.tensor_scalar(m3[:], m3[:], -1.0, 3.0, OP.mult, OP.add)
                E.tensor_scalar(MK[k][:], m3[:], 0.0, None, OP.is_gt)
                E.tensor_scalar(m3[:], m3[:], 1e-12, None, OP.max)

                hcl = pool.tile([P, X], F32, tag="hcl")
                S.activation(hcl[:], m3[:], AF.Ln)
                S.activation(hcl[:], hcl[:], AF.Exp, scale=0.5)

                # hg = hcl * ||d|| * rinv
                E.tensor_tensor(HG[k][:], ndc[:], rinv[:], OP.mult)
                E.tensor_tensor(HG[k][:], HG[k][:], hcl[:], OP.mult)

                htd = pool.tile([P, 3, X], F32, tag="htd")
                for j in range(3):
                    E.tensor_tensor(htd[:, j, :], hcl[:], td[:, j, :], OP.mult)

                # PL slots 0..9: cen + t_s*htd ; slot 10: 1.5*u - rp/s
                PL = pool.tile([P, NS + 1, 3, X], F32, tag="PL", bufs=3)
                for si in range(NS):
                    E.scalar_tensor_tensor(PL[:, si, :, :], htd[:, :, :], _f(t[si]),
                                           cen[:, :, :], OP.mult, OP.add)
                for j in range(3):
                    E.tensor_scalar(PL[:, NS, j, :], u[:, j, :], 1.5, _f(-rps[j]),
                                    OP.mult, OP.add)

                # dt0 = ||PL0*s + rp|| ; dt10 = ||(PL10-PL9)*s||
                q3 = pool.tile([P, 3, X], F32, tag="q3")
                for j in range(3):
                    S.activation(q3[:, j, :], PL[:, 0, j, :], AF.Square,
                                 bias=_f(rp[j]), scale=_f(s[j]))
                dtt = DTT[k]
                E.tensor_tensor(dtt[:, 0, :], q3[:, 0, :], q3[:, 1, :], OP.add)
                E.tensor_tensor(dtt[:, 0, :], dtt[:, 0, :], q3[:, 2, :], OP.add)

                df = pool.tile([P, 3, X], F32, tag="q3b")
                E.tensor_tensor(df[:, :, :], PL[:, NS, :, :], PL[:, NS - 1, :, :], OP.subtract)
                for j in range(3):
                    S.activation(df[:, j, :], df[:, j, :], AF.Square, scale=_f(s[j]))
                E.tensor_tensor(dtt[:, 1, :], df[:, 0, :], df[:, 1, :], OP.add)
                E.tensor_tensor(dtt[:, 1, :], dtt[:, 1, :], df[:, 2, :], OP.add)
                S.activation(dtt[:, :, :], dtt[:, :, :], AF.Ln)
                S.activation(dtt[:, :, :], dtt[:, :, :], AF.Exp, scale=0.5)

                # F chain, in place over PL
                flat = PL[:, :, :, :]
                S.activation(flat, flat, AF.Abs)                       # |PL|
                S.activation(flat, flat, AF.Ln, bias=_f(EPS))          # ln(|PL|+eps)
                S.activation(PL[:, :, 0:2, :], PL[:, :, 0:2, :], AF.Exp,
                             scale=_f(cc["c1"]))                       # u,v
                E.tensor_tensor(PL[:, :, 0, :], PL[:, :, 0, :], PL[:, :, 1, :], OP.add)
                S.activation(PL[:, :, 0, :], PL[:, :, 0, :], AF.Ln)
                S.activation(PL[:, :, 0, :], PL[:, :, 0, :], AF.Exp, scale=_f(cc["c2"]))
                S.activation(PL[:, :, 2, :], PL[:, :, 2, :], AF.Exp, scale=_f(cc["c3"]))
                E.tensor_tensor(PL[:, :, 0, :], PL[:, :, 0, :], PL[:, :, 2, :], OP.add)
                S.activation(PL[:, :, 0, :], PL[:, :, 0, :], AF.Ln)
                S.activation(FF[k][:, :, :], PL[:, :, 0, :], AF.Exp, scale=_f(cc["e1"]))

        tc.no_sync_barrier()

        # ---------------- phase 2: occupancy/visibility (exp set) -----------
        with tc.tile_pool(name="p2", bufs=3) as pool:
            for k in live:
                cc = consts[k]
                E = V
                lr0, nr, j0, nj = rects[k]
                X = nr * nj
                th = pool.tile([P, NS + 1, X], F32, tag="th", bufs=3)
                # occ = 0.5 + 0.5*tanh(500*(1-F)) == sigmoid(1000*(1-F))
                S.activation(th[:, :, :], FF[k][:, :, :], AF.Tanh,
                             bias=SHARP / 2.0, scale=-SHARP / 2.0)
                E.tensor_scalar(th[:, :, :], th[:, :, :], 0.5, 0.5, OP.mult, OP.add)

                cum = pool.tile([P, NS + 1, X], F32, tag="cum", bufs=3)
                E.tensor_scalar(cum[:, 0, :], th[:, 0, :], _f(cc["occ0"]), None, OP.add)
                for si in range(1, NS + 1):
                    E.tensor_tensor(cum[:, si, :], cum[:, si - 1, :], th[:, si, :], OP.add)
                S.activation(cum[:, :, :], cum[:, :, :], AF.Exp, scale=-TAU)  # v_1..v_11

                acc = pool.tile([P, X], F32, tag="acc")
                E.tensor_scalar(acc[:], cum[:, 0, :], _f(beta[1]), None, OP.mult)
                for si in range(1, NS):
                    E.scalar_tensor_tensor(acc[:], cum[:, si, :], _f(beta[si + 1]),
                                           acc[:], OP.mult, OP.add)
                E.tensor_tensor(acc[:], acc[:], HG[k][:], OP.mult)

                b1 = pool.tile([P, X], F32, tag="b1")
                E.tensor_scalar(b1[:], cum[:, 0, :], 0.5, _f(0.5 * cc["vis0"]),
                                OP.mult, OP.add)
                E.tensor_tensor(b1[:], b1[:], DTT[k][:, 0, :], OP.mult)
                E.tensor_tensor(acc[:], acc[:], b1[:], OP.add)

                b2 = pool.tile([P, X], F32, tag="b2")
                E.tensor_tensor(b2[:], cum[:, NS - 1, :], cum[:, NS, :], OP.add)
                E.scalar_tensor_tensor(b2[:], b2[:], 0.5, DTT[k][:, 1, :], OP.mult, OP.mult)
                E.tensor_tensor(acc[:], acc[:], b2[:], OP.add)

                # dmin[rect] = min(dmin[rect], mask ? depth : FAR)
                E.tensor_scalar(acc[:], acc[:], 1.0, -FAR, OP.mult, OP.add)
                E.tensor_tensor(acc[:], acc[:], MK[k][:], OP.mult)
                E.tensor_scalar(acc[:], acc[:], FAR, None, OP.add)
                dv = dmin[:, lr0:lr0 + nr, j0:j0 + nj]
                V.tensor_tensor(dv, dv, acc[:].rearrange("p (a b) -> p a b", b=nj),
                                OP.min)

        nc.sync.dma_start(out_dram.ap(), dmin[:, :, :])

    # Pre-place the two ACT table loads (natural_log_exp for phase 1,
    # exp_and_others for phase 2/tanh) so bacc's fixpoint inserts none.
    # (CoreSim can't handle the hand-inserted loads; act_loads=False skips.)
    if not act_loads:
        nc.compile()
        return nc
    from concourse.hw_specs import get_activation_tables
    names = list(get_activation_tables(nc.m.arch).keys())
    id_nle = names.index("natural_log_exp_and_others")
    id_exp = names.index("exp_and_others")

    def make_load(set_id):
        ins = mybir.InstLoadActFuncSet(
            name=nc.get_next_instruction_name(), act_func_set_id=set_id,
            ins=[], outs=[])
        ins.engine = nc.scalar.engine
        return ins

    for blk in nc.main_func.blocks:
        il = blk.instructions
        first_act = next((i for i, x in enumerate(il)
                          if isinstance(x, mybir.InstActivation)), None)
        if first_act is None:
            continue
        first_tanh = next((i for i, x in enumerate(il)
                           if isinstance(x, mybir.InstActivation)
                           and x.func == AF.Tanh), None)
        il.insert(first_act, make_load(id_nle))
        if first_tanh is not None:
            il.insert(first_tanh + 1, make_load(id_exp))

    nc.compile()
    return nc


def _shard_rays(rays_d):
    """-> per-core arrays [128, 3, 45, 5]; core c owns rows 8*lr+c."""
    rd = np.asarray(rays_d, np.float32)
    out = []
    for c in range(N_CORES):
        sub = rd[c::N_CORES]                         # (45, 640, 3)
        arr = sub.reshape(NRL, NJ, 128, 3).transpose(2, 3, 0, 1)
        out.append(np.ascontiguousarray(arr))        # (128, 3, 45, 5)
    return out


def _unshard(outs):
    """outs: list of 8 arrays [128, 45, 5] -> (360, 640)."""
    full = np.empty((HS, WS), np.float32)
    for c in range(N_CORES):
        full[c::N_CORES] = outs[c].transpose(1, 2, 0).reshape(NRL, WS)
    return full


def kernel(sq_poses, sq_params, rays_d, rays_o, t, **run_kwargs):
    consts, tv, beta = _host_consts(sq_poses, sq_params, rays_o, t)
    rects = _host_rects(consts, rays_d)
    nc = build_program(consts, tv, beta, rects)
    planes = _shard_rays(rays_d)
    in_maps = [{"rdin": planes[c]} for c in range(N_CORES)]
    res = run_bass_kernel_spmd(nc, in_maps, core_ids=list(range(N_CORES)), **run_kwargs)
    outs = [res.results[c]["depth"] for c in range(N_CORES)]
    out = _unshard(outs).astype(np.float32)
    kernel.last_result = res
    return out


kernel.last_result = None


# revision 27
# speedup vs baseline: 1.0872x; 1.0431x over previous
"""Trainium2 Bass kernel for nn_DepthRenderer (superquadric depth renderer).

Sharding: rows round-robin over 8 cores (core c owns image rows r = 8*lr+c,
lr=0..44).  Per-core layout [128 lanes, 45 lrows, 5 xblocks]; lane = x%128,
xblock = x//128.  Each core renders all 8 SQs (constants baked as immediates
into one SPMD program) and min-accumulates depth on device; host concatenates.

Sparsity: a SQ can only influence pixels where the ray enters its bounding
sphere: h(d) = (b.d)^2 - (C-3) * d^T A d > 0 (homogeneous quadratic in the ray
direction, so normalization-free).  The host evaluates h on a coarse pixel
subgrid, takes the bounding rectangle (+margin, rows rounded to multiples of 8
so the rect is the SAME static view on every core), and the device program
processes only that rect per SQ (~4.7x less work).  Pixels outside the rect
keep depth FAR; rect pixels use the exact in-rect mask, and the premask
boundary is depth-continuous (grazing rays integrate to ~FAR), so the coarse
rect is safe.

Math notes (exact rewrites of the reference, up to fp rounding):
  - a == sizes  =>  X = |loc|/a + eps = |pts_loc| + eps  (sizes cancel)
  - ||td * sizes|| = ||d|| * rinv  (rotation invariance)
  - dt0  = ||pts_loc[0]*s + R^T p||,  dt10 = ||(PL10-PL9)*s||  with
    PL10 = loc_far/s = 1.5*u - (R^T p)/s
  - sqrt(x) = exp(0.5*ln(x)); sigmoid(x) = 0.5 + 0.5*tanh(x/2)
  - phase 1 (pow chains) uses the natural_log_exp ACT table set, phase 2
    (tanh occupancy + visibility exp) uses exp_and_others; both loads are
    pre-placed so bacc inserts no further table switches.
"""

from contextlib import ExitStack

import numpy as np

import concourse.bass as bass
import concourse.bacc as bacc
import concourse.mybir as mybir
from concourse import tile
from concourse.bass_utils import run_bass_kernel_spmd

F32 = mybir.dt.float32
AF = mybir.ActivationFunctionType
OP = mybir.AluOpType

# renderer constants (match the nn.Module init)
HS, WS = 360, 640
NEAR, FAR = 0.0, 1.5
NS = 10
SHARP = 1000.0
TAU = 100.0
N_SQ = 8
EPS = 1e-6

N_CORES = 8
NRL = HS // N_CORES       # 45 local rows per core
NJ = WS // 128            # 5 x-blocks
NCOL = NRL * NJ           # 225 columns per core
P = 128


def _f(x):
    return float(np.float32(x))


def _host_consts(sq_poses, sq_params, rays_o, t):
    """Per-SQ scalars, computed in float64 from the f32 inputs."""
    sq_poses = np.asarray(sq_poses, np.float64)
    sq_params = np.asarray(sq_params, np.float64)
    rays_o = np.asarray(rays_o, np.float64)
    t = np.asarray(t, np.float64)

    consts = []
    for k in range(N_SQ):
        R = sq_poses[k, :3, :3]
        p = sq_poses[k, :3, 3]
        s = sq_params[k, 0:3]
        e1 = sq_params[k, 3]
        e2 = sq_params[k, 4]

        M1 = R.T / s[:, None]            # u = M1 @ d = (R^T d)/s
        tc = (R.T @ (rays_o - p)) / s
        rp = R.T @ p                      # loc(near) = -rp
        rps = rp / s
        c1 = 2.0 / e2
        c2 = e2 / e1
        c3 = 2.0 / e1

        # near-point occupancy (constant per SQ)
        Xn = np.abs(-rp) / s + EPS
        fN = (Xn[0] ** c1 + Xn[1] ** c1) ** c2 + Xn[2] ** c3
        Fn = fN ** e1
        with np.errstate(over="ignore"):
            occ0 = 1.0 / (1.0 + np.exp(-SHARP * (1.0 - Fn)))
        vis0 = np.exp(-TAU * occ0)

        consts.append(dict(
            M1=M1, tc=tc, rp=rp, rps=rps, s=s,
            c1=c1, c2=c2, c3=c3, e1=e1,
            occ0=occ0, vis0=vis0,
        ))

    # segment weights from t (shared across SQs)
    dt_abs = np.abs(np.diff(t))          # |t_i - t_{i-1}|, i=1..9
    beta = np.zeros(11)                  # weight of v_s (s=1..10) in inner sum
    for i in range(1, NS):               # inner gaps i=1..9 use v_i, v_{i+1}
        beta[i] += 0.5 * dt_abs[i - 1]
        beta[i + 1] += 0.5 * dt_abs[i - 1]
    return consts, t, beta


def _host_rects(consts, rays_d):
    """Per-SQ (lr0, nr, j0, nj) bounding rect, identical across cores.

    h(d) = (b.d)^2 - (C-3) d^T A d is degree-2 homogeneous in d, so the
    coarse-subgrid sign test needs no ray normalization.  Conservative by a
    9px margin (>> 3px grid step; min blob diameter is ~40px for any SQ with
    C comfortably > 3).  Rows rounded to multiples of 8 so that every core's
    local-row range is the same [lr0, lr0+nr).
    """
    d = np.asarray(rays_d, np.float64)
    ys = np.arange(0, HS, 3)
    xs = np.arange(0, WS, 3)
    sub = d[np.ix_(ys, xs)]
    rects = []
    for cc in consts:
        M1, tcv = cc["M1"], cc["tc"]
        C = float((tcv ** 2).sum())
        if C <= 3.5:                      # near/inside bounding sphere: dense
            rects.append((0, NRL, 0, NJ))
            continue
        A = M1.T @ M1
        b = M1.T @ tcv
        hq = (sub @ b) ** 2 - (C - 3.0) * np.einsum("yxi,ij,yxj->yx", sub, A, sub)
        hit = hq > 0
        if not hit.any():
            rects.append(None)
            continue
        ryy, rxx = np.where(hit)
        r0 = max(0, int(ys[ryy.min()]) - 9)
        r1 = min(HS - 1, int(ys[ryy.max()]) + 9)
        x0 = max(0, int(xs[rxx.min()]) - 9)
        x1 = min(WS - 1, int(xs[rxx.max()]) + 9)
        r0 = (r0 // 8) * 8
        r1 = min(HS, ((r1 + 8) // 8) * 8) - 1
        lr0, nr = r0 // 8, (r1 - r0 + 1) // 8
        j0, j1 = x0 // 128, x1 // 128
        rects.append((lr0, nr, j0, j1 - j0 + 1))
    return rects


def build_program(consts, t, beta, rects, act_loads=True):
    """One SPMD program; input rdin [128,3,45,5], output depth [128,45,5]."""
    nc = bacc.Bacc("TRN2", target_bir_lowering=False, debug=False,
                   enable_asserts=False, num_devices=N_CORES)

    rd_dram = nc.dram_tensor("rdin", [P, 3, NRL, NJ], F32, kind="ExternalInput")
    out_dram = nc.dram_tensor("depth", [P, NRL, NJ], F32, kind="ExternalOutput")

    # const APs for activation biases (only 0.0/1.0 are pre-registered)
    def reg_const(v):
        v = _f(v)
        if (F32, v) not in nc.const_aps.aps:
            th = nc.alloc_sbuf_tensor(f"constap{len(nc.const_aps.aps)}", [128, 1], F32)
            nc.gpsimd.memset(th.ap(), v)
            nc.const_aps.aps[(F32, v)] = th.ap()

    reg_const(EPS)
    reg_const(-SHARP)
    for cc in consts:
        for j in range(3):
            reg_const(cc["rp"][j])
    nc.all_engine_barrier()

    live = [k for k in range(N_SQ) if rects[k] is not None]

    with tile.TileContext(nc) as tc, ExitStack() as es:
        V = nc.vector
        S = nc.scalar
        persist = es.enter_context(tc.tile_pool(name="persist", bufs=1))

        # ---- shared loads & per-core shared prep ----
        rd = persist.tile([P, 3, NRL, NJ], F32, name="rd")
        nc.sync.dma_start(rd[:, :, :, :], rd_dram.ap())

        rdsq = persist.tile([P, 3, NRL, NJ], F32, name="rdsq")
        S.activation(rdsq[:, :, :, :], rd[:, :, :, :], AF.Square)
        nd2 = persist.tile([P, NRL, NJ], F32, name="nd2")
        V.tensor_tensor(nd2[:, :, :], rdsq[:, 0, :, :], rdsq[:, 1, :, :], OP.add)
        V.tensor_tensor(nd2[:, :, :], nd2[:, :, :], rdsq[:, 2, :, :], OP.add)
        nd = persist.tile([P, NRL, NJ], F32, name="nd")
        S.activation(nd[:, :, :], nd2[:, :, :], AF.Ln)
        S.activation(nd[:, :, :], nd[:, :, :], AF.Exp, scale=0.5)

        dmin = persist.tile([P, NRL, NJ], F32, name="dmin")
        V.memset(dmin[:, :, :], FAR)

        # persistent per-SQ results for phase 2 (sized per rect)
        FF, MK, HG, DTT = {}, {}, {}, {}
        for k in live:
            lr0, nr, j0, nj = rects[k]
            X = nr * nj
            FF[k] = persist.tile([P, NS + 1, X], F32, name=f"FF{k}")
            MK[k] = persist.tile([P, X], F32, name=f"MK{k}")
            HG[k] = persist.tile([P, X], F32, name=f"HG{k}")
            DTT[k] = persist.tile([P, 2, X], F32, name=f"DTT_{k}")

        # ---------------- phase 1: per-SQ F chains (ln/exp table set) -------
        with tc.tile_pool(name="p1", bufs=3) as pool:
            for k in live:
                cc = consts[k]
                E = V
                lr0, nr, j0, nj = rects[k]
                X = nr * nj
                M1, tcv, rp, rps, s = cc["M1"], cc["tc"], cc["rp"], cc["rps"], cc["s"]

                def r4(ap2):   # [P, X] compact view -> [P, nr, nj]
                    return ap2.rearrange("p (a b) -> p a b", b=nj)

                # compact copies of the rect slice of rd (and ||d||)
                rdc = pool.tile([P, 3, X], F32, tag="rdc")
                E.tensor_copy(rdc[:, :, :].rearrange("p c (a b) -> p c a b", b=nj),
                              rd[:, :, lr0:lr0 + nr, j0:j0 + nj])
                ndc = pool.tile([P, X], F32, tag="ndc")
                E.tensor_copy(r4(ndc[:, :]), nd[:, lr0:lr0 + nr, j0:j0 + nj])

                u = pool.tile([P, 3, X], F32, tag="u")
                for j in range(3):
                    E.tensor_scalar(u[:, j, :], rdc[:, 0, :], _f(M1[j, 0]), None, OP.mult)
                    E.scalar_tensor_tensor(u[:, j, :], rdc[:, 1, :], _f(M1[j, 1]), u[:, j, :], OP.mult, OP.add)
                    E.scalar_tensor_tensor(u[:, j, :], rdc[:, 2, :], _f(M1[j, 2]), u[:, j, :], OP.mult, OP.add)

                usq = pool.tile([P, 3, X], F32, tag="usq")
                E.tensor_tensor(usq[:, :, :], u[:, :, :], u[:, :, :], OP.mult)
                nu2 = pool.tile([P, X], F32, tag="nu2")
                E.tensor_tensor(nu2[:], usq[:, 0, :], usq[:, 1, :], OP.add)
                E.tensor_tensor(nu2[:], nu2[:], usq[:, 2, :], OP.add)

                rinv = pool.tile([P, X], F32, tag="rinv")
                S.activation(rinv[:], nu2[:], AF.Ln)
                S.activation(rinv[:], rinv[:], AF.Exp, scale=-0.5)

                td = pool.tile([P, 3, X], F32, tag="td")
                for j in range(3):
                    E.tensor_tensor(td[:, j, :], u[:, j, :], rinv[:], OP.mult)

                d1 = pool.tile([P, X], F32, tag="d1")
                E.tensor_scalar(d1[:], td[:, 0, :], _f(tcv[0]), None, OP.mult)
                E.scalar_tensor_tensor(d1[:], td[:, 1, :], _f(tcv[1]), d1[:], OP.mult, OP.add)
                E.scalar_tensor_tensor(d1[:], td[:, 2, :], _f(tcv[2]), d1[:], OP.mult, OP.add)
                proj = pool.tile([P, X], F32, tag="proj")
                S.activation(proj[:], d1[:], AF.Abs)

                cen = pool.tile([P, 3, X], F32, tag="cen")
                for j in range(3):
                    E.tensor_tensor(cen[:, j, :], proj[:], td[:, j, :], OP.mult)
                    E.tensor_scalar(cen[:, j, :], cen[:, j, :], _f(tcv[j]), None, OP.add)

                csq = pool.tile([P, 3, X], F32, tag="usq")
                E.tensor_tensor(csq[:, :, :], cen[:, :, :], cen[:, :, :], OP.mult)
                m3 = pool.tile([P, X], F32, tag="m3")
                E.tensor_tensor(m3[:], csq[:, 0, :], csq[:, 1, :], OP.add)
                E.tensor_tensor(m3[:], m3[:], csq[:, 2, :], OP.add)
                # m3 = 3 - dist^2 ; mask = m3 > 0 ; hclsq = max(m3, 1e-12)
                E.tensor_scalar(m3[:], m3[:], -1.0, 3.0, OP.mult, OP.add)
                E.tensor_scalar(MK[k][:], m3[:], 0.0, None, OP.is_gt)
                E.tensor_scalar(m3[:], m3[:], 1e-12, None, OP.max)

                hcl = pool.tile([P, X], F32, tag="hcl")
                S.activation(hcl[:], m3[:], AF.Ln)
                S.activation(hcl[:], hcl[:], AF.Exp, scale=0.5)

                # hg = hcl * ||d|| * rinv
                E.tensor_tensor(HG[k][:], ndc[:], rinv[:], OP.mult)
                E.tensor_tensor(HG[k][:], HG[k][:], hcl[:], OP.mult)

                htd = pool.tile([P, 3, X], F32, tag="htd")
                for j in range(3):
                    E.tensor_tensor(htd[:, j, :], hcl[:], td[:, j, :], OP.mult)

                # PL slots 0..9: cen + t_s*htd ; slot 10: 1.5*u - rp/s
                PL = pool.tile([P, NS + 1, 3, X], F32, tag="PL", bufs=3)
                for si in range(NS):
                    E.scalar_tensor_tensor(PL[:, si, :, :], htd[:, :, :], _f(t[si]),
                                           cen[:, :, :], OP.mult, OP.add)
                for j in range(3):
                    E.tensor_scalar(PL[:, NS, j, :], u[:, j, :], 1.5, _f(-rps[j]),
                                    OP.mult, OP.add)

                # dt0 = ||PL0*s + rp|| ; dt10 = ||(PL10-PL9)*s||
                q3 = pool.tile([P, 3, X], F32, tag="q3")
                for j in range(3):
                    S.activation(q3[:, j, :], PL[:, 0, j, :], AF.Square,
                                 bias=_f(rp[j]), scale=_f(s[j]))
                dtt = DTT[k]
                E.tensor_tensor(dtt[:, 0, :], q3[:, 0, :], q3[:, 1, :], OP.add)
                E.tensor_tensor(dtt[:, 0, :], dtt[:, 0, :], q3[:, 2, :], OP.add)

                df = pool.tile([P, 3, X], F32, tag="q3b")
                E.tensor_tensor(df[:, :, :], PL[:, NS, :, :], PL[:, NS - 1, :, :], OP.subtract)
                for j in range(3):
                    S.activation(df[:, j, :], df[:, j, :], AF.Square, scale=_f(s[j]))
                E.tensor_tensor(dtt[:, 1, :], df[:, 0, :], df[:, 1, :], OP.add)
                E.tensor_tensor(dtt[:, 1, :], dtt[:, 1, :], df[:, 2, :], OP.add)
                S.activation(dtt[:, :, :], dtt[:, :, :], AF.Ln)
                S.activation(dtt[:, :, :], dtt[:, :, :], AF.Exp, scale=0.5)

                # F chain, in place over PL
                flat = PL[:, :, :, :]
                S.activation(flat, flat, AF.Abs)                       # |PL|
                S.activation(flat, flat, AF.Ln, bias=_f(EPS))          # ln(|PL|+eps)
                S.activation(PL[:, :, 0:2, :], PL[:, :, 0:2, :], AF.Exp,
                             scale=_f(cc["c1"]))                       # u,v
                E.tensor_tensor(PL[:, :, 0, :], PL[:, :, 0, :], PL[:, :, 1, :], OP.add)
                S.activation(PL[:, :, 0, :], PL[:, :, 0, :], AF.Ln)
                S.activation(PL[:, :, 0, :], PL[:, :, 0, :], AF.Exp, scale=_f(cc["c2"]))
                S.activation(PL[:, :, 2, :], PL[:, :, 2, :], AF.Exp, scale=_f(cc["c3"]))
                E.tensor_tensor(PL[:, :, 0, :], PL[:, :, 0, :], PL[:, :, 2, :], OP.add)
                S.activation(PL[:, :, 0, :], PL[:, :, 0, :], AF.Ln)
                S.activation(FF[k][:, :, :], PL[:, :, 0, :], AF.Exp, scale=_f(cc["e1"]))

                # ---- occupancy/visibility/depth (same ln/exp table set) ----
                # occ = sigmoid(1000*(1-F)) = 1/(1 + e^(1000F-1000)); F clamped
                # at 1.088 so e^x <= 1.65e38 (sigma there is 6e-39 ~ 0).
                occ = pool.tile([P, NS + 1, X], F32, tag="occ", bufs=3)
                E.tensor_scalar(FF[k][:, :, :], FF[k][:, :, :], 1.088, None, OP.min)
                S.activation(occ[:, :, :], FF[k][:, :, :], AF.Exp,
                             scale=SHARP, bias=-SHARP)
                E.tensor_scalar(occ[:, :, :], occ[:, :, :], 1.0, None, OP.add)
                rscr = pool.tile([P, NS + 1, X], F32, tag="rscr")
                E.reciprocal_approx_fast(rscr[:, :, :], occ[:, :, :])

                cum = pool.tile([P, NS + 1, X], F32, tag="cum", bufs=3)
                E.tensor_scalar(cum[:, 0, :], rscr[:, 0, :], _f(cc["occ0"]), None, OP.add)
                for si in range(1, NS + 1):
                    E.tensor_tensor(cum[:, si, :], cum[:, si - 1, :], rscr[:, si, :], OP.add)
                S.activation(cum[:, :, :], cum[:, :, :], AF.Exp, scale=-TAU)  # v_1..v_11

                acc = pool.tile([P, X], F32, tag="acc")
                E.tensor_scalar(acc[:], cum[:, 0, :], _f(beta[1]), None, OP.mult)
                for si in range(1, NS):
                    E.scalar_tensor_tensor(acc[:], cum[:, si, :], _f(beta[si + 1]),
                                           acc[:], OP.mult, OP.add)
                E.tensor_tensor(acc[:], acc[:], HG[k][:], OP.mult)

                b1 = pool.tile([P, X], F32, tag="b1")
                E.tensor_scalar(b1[:], cum[:, 0, :], 0.5, _f(0.5 * cc["vis0"]),
                                OP.mult, OP.add)
                E.tensor_tensor(b1[:], b1[:], DTT[k][:, 0, :], OP.mult)
                E.tensor_tensor(acc[:], acc[:], b1[:], OP.add)

                b2 = pool.tile([P, X], F32, tag="b2")
                E.tensor_tensor(b2[:], cum[:, NS - 1, :], cum[:, NS, :], OP.add)
                E.scalar_tensor_tensor(b2[:], b2[:], 0.5, DTT[k][:, 1, :], OP.mult, OP.mult)
                E.tensor_tensor(acc[:], acc[:], b2[:], OP.add)

                # dmin[rect] = min(dmin[rect], mask ? depth : FAR)
                E.tensor_scalar(acc[:], acc[:], 1.0, -FAR, OP.mult, OP.add)
                E.tensor_tensor(acc[:], acc[:], MK[k][:], OP.mult)
                E.tensor_scalar(acc[:], acc[:], FAR, None, OP.add)
                dv = dmin[:, lr0:lr0 + nr, j0:j0 + nj]
                V.tensor_tensor(dv, dv, acc[:].rearrange("p (a b) -> p a b", b=nj),
                                OP.min)


        nc.sync.dma_start(out_dram.ap(), dmin[:, :, :])

    # Pre-place the two ACT table loads (natural_log_exp for phase 1,
    # exp_and_others for phase 2/tanh) so bacc's fixpoint inserts none.
    # (CoreSim can't handle the hand-inserted loads; act_loads=False skips.)
    if not act_loads:
        nc.compile()
        return nc
    from concourse.hw_specs import get_activation_tables
    names = list(get_activation_tables(nc.m.arch).keys())
    id_nle = names.index("natural_log_exp_and_others")

    def make_load(set_id):
        ins = mybir.InstLoadActFuncSet(
            name=nc.get_next_instruction_name(), act_func_set_id=set_id,
            ins=[], outs=[])
        ins.engine = nc.scalar.engine
        return ins

    for blk in nc.main_func.blocks:
        il = blk.instructions
        first_act = next((i for i, x in enumerate(il)
                          if isinstance(x, mybir.InstActivation)), None)
        if first_act is None:
            continue
        il.insert(first_act, make_load(id_nle))

    nc.compile()
    return nc


def _shard_rays(rays_d):
    """-> per-core arrays [128, 3, 45, 5]; core c owns rows 8*lr+c."""
    rd = np.asarray(rays_d, np.float32)
    out = []
    for c in range(N_CORES):
        sub = rd[c::N_CORES]                         # (45, 640, 3)
        arr = sub.reshape(NRL, NJ, 128, 3).transpose(2, 3, 0, 1)
        out.append(np.ascontiguousarray(arr))        # (128, 3, 45, 5)
    return out


def _unshard(outs):
    """outs: list of 8 arrays [128, 45, 5] -> (360, 640)."""
    full = np.empty((HS, WS), np.float32)
    for c in range(N_CORES):
        full[c::N_CORES] = outs[c].transpose(1, 2, 0).reshape(NRL, WS)
    return full


def kernel(sq_poses, sq_params, rays_d, rays_o, t, **run_kwargs):
    consts, tv, beta = _host_consts(sq_poses, sq_params, rays_o, t)
    rects = _host_rects(consts, rays_d)
    nc = build_program(consts, tv, beta, rects)
    planes = _shard_rays(rays_d)
    in_maps = [{"rdin": planes[c]} for c in range(N_CORES)]
    res = run_bass_kernel_spmd(nc, in_maps, core_ids=list(range(N_CORES)), **run_kwargs)
    outs = [res.results[c]["depth"] for c in range(N_CORES)]
    out = _unshard(outs).astype(np.float32)
    kernel.last_result = res
    return out


kernel.last_result = None


# revision 28
# speedup vs baseline: 1.1457x; 1.0539x over previous
"""Trainium2 Bass kernel for nn_DepthRenderer (superquadric depth renderer).

Sharding: rows round-robin over 8 cores (core c owns image rows r = 8*lr+c,
lr=0..44).  Per-core layout [128 lanes, 45 lrows, 5 xblocks]; lane = x%128,
xblock = x//128.  Each core renders all 8 SQs (constants baked as immediates
into one SPMD program) and min-accumulates depth on device; host concatenates.

Sparsity: a SQ can only influence pixels where the ray enters its bounding
sphere: h(d) = (b.d)^2 - (C-3) * d^T A d > 0 (homogeneous quadratic in the ray
direction, so normalization-free).  The host evaluates h on a coarse pixel
subgrid, takes the bounding rectangle (+margin, rows rounded to multiples of 8
so the rect is the SAME static view on every core), and the device program
processes only that rect per SQ (~4.7x less work).  Pixels outside the rect
keep depth FAR; rect pixels use the exact in-rect mask, and the premask
boundary is depth-continuous (grazing rays integrate to ~FAR), so the coarse
rect is safe.

Math notes (exact rewrites of the reference, up to fp rounding):
  - a == sizes  =>  X = |loc|/a + eps = |pts_loc| + eps  (sizes cancel)
  - ||td * sizes|| = ||d|| * rinv  (rotation invariance)
  - dt0  = ||pts_loc[0]*s + R^T p||,  dt10 = ||(PL10-PL9)*s||  with
    PL10 = loc_far/s = 1.5*u - (R^T p)/s
  - sqrt(x) = exp(0.5*ln(x)); sigmoid(x) = 0.5 + 0.5*tanh(x/2)
  - phase 1 (pow chains) uses the natural_log_exp ACT table set, phase 2
    (tanh occupancy + visibility exp) uses exp_and_others; both loads are
    pre-placed so bacc inserts no further table switches.
"""

from contextlib import ExitStack

import numpy as np

import concourse.bass as bass
import concourse.bacc as bacc
import concourse.mybir as mybir
from concourse import tile
from concourse.bass_utils import run_bass_kernel_spmd

F32 = mybir.dt.float32
AF = mybir.ActivationFunctionType
OP = mybir.AluOpType

# renderer constants (match the nn.Module init)
HS, WS = 360, 640
NEAR, FAR = 0.0, 1.5
NS = 10
SHARP = 1000.0
TAU = 100.0
N_SQ = 8
EPS = 1e-6

N_CORES = 8
NRL = HS // N_CORES       # 45 local rows per core
NJ = WS // 128            # 5 x-blocks
NCOL = NRL * NJ           # 225 columns per core
P = 128


def _f(x):
    return float(np.float32(x))


def _host_consts(sq_poses, sq_params, rays_o, t):
    """Per-SQ scalars, computed in float64 from the f32 inputs."""
    sq_poses = np.asarray(sq_poses, np.float64)
    sq_params = np.asarray(sq_params, np.float64)
    rays_o = np.asarray(rays_o, np.float64)
    t = np.asarray(t, np.float64)

    consts = []
    for k in range(N_SQ):
        R = sq_poses[k, :3, :3]
        p = sq_poses[k, :3, 3]
        s = sq_params[k, 0:3]
        e1 = sq_params[k, 3]
        e2 = sq_params[k, 4]

        M1 = R.T / s[:, None]            # u = M1 @ d = (R^T d)/s
        tc = (R.T @ (rays_o - p)) / s
        rp = R.T @ p                      # loc(near) = -rp
        rps = rp / s
        c1 = 2.0 / e2
        c2 = e2 / e1
        c3 = 2.0 / e1

        # near-point occupancy (constant per SQ)
        Xn = np.abs(-rp) / s + EPS
        fN = (Xn[0] ** c1 + Xn[1] ** c1) ** c2 + Xn[2] ** c3
        Fn = fN ** e1
        with np.errstate(over="ignore"):
            occ0 = 1.0 / (1.0 + np.exp(-SHARP * (1.0 - Fn)))
        vis0 = np.exp(-TAU * occ0)

        consts.append(dict(
            M1=M1, tc=tc, rp=rp, rps=rps, s=s,
            c1=c1, c2=c2, c3=c3, e1=e1,
            occ0=occ0, vis0=vis0,
        ))

    # segment weights from t (shared across SQs)
    dt_abs = np.abs(np.diff(t))          # |t_i - t_{i-1}|, i=1..9
    beta = np.zeros(11)                  # weight of v_s (s=1..10) in inner sum
    for i in range(1, NS):               # inner gaps i=1..9 use v_i, v_{i+1}
        beta[i] += 0.5 * dt_abs[i - 1]
        beta[i + 1] += 0.5 * dt_abs[i - 1]
    return consts, t, beta


def _host_rects(consts, rays_d):
    """Per-SQ (lr0, nr, j0, nj) bounding rect, identical across cores.

    h(d) = (b.d)^2 - (C-3) d^T A d is degree-2 homogeneous in d, so the
    coarse-subgrid sign test needs no ray normalization.  Conservative by a
    9px margin (>> 3px grid step; min blob diameter is ~40px for any SQ with
    C comfortably > 3).  Rows rounded to multiples of 8 so that every core's
    local-row range is the same [lr0, lr0+nr).
    """
    d = np.asarray(rays_d, np.float64)
    ys = np.arange(0, HS, 2)
    xs = np.arange(0, WS, 2)
    sub = d[np.ix_(ys, xs)]
    rects = []
    for cc in consts:
        M1, tcv = cc["M1"], cc["tc"]
        C = float((tcv ** 2).sum())
        if C <= 3.5:                      # near/inside bounding sphere: dense
            rects.append((0, NRL, 0, NJ))
            continue
        A = M1.T @ M1
        b = M1.T @ tcv
        hq = (sub @ b) ** 2 - (C - 3.0) * np.einsum("yxi,ij,yxj->yx", sub, A, sub)
        hit = hq > 0
        if not hit.any():
            rects.append(None)
            continue
        ryy, rxx = np.where(hit)
        r0 = max(0, int(ys[ryy.min()]) - 5)
        r1 = min(HS - 1, int(ys[ryy.max()]) + 5)
        x0 = max(0, int(xs[rxx.min()]) - 5)
        x1 = min(WS - 1, int(xs[rxx.max()]) + 5)
        r0 = (r0 // 8) * 8
        r1 = min(HS, ((r1 + 8) // 8) * 8) - 1
        lr0, nr = r0 // 8, (r1 - r0 + 1) // 8
        j0, j1 = x0 // 128, x1 // 128
        rects.append((lr0, nr, j0, j1 - j0 + 1))
    return rects


def build_program(consts, t, beta, rects, act_loads=True):
    """One SPMD program; input rdin [128,3,45,5], output depth [128,45,5]."""
    nc = bacc.Bacc("TRN2", target_bir_lowering=False, debug=False,
                   enable_asserts=False, num_devices=N_CORES)

    rd_dram = nc.dram_tensor("rdin", [P, 3, NRL, NJ], F32, kind="ExternalInput")
    out_dram = nc.dram_tensor("depth", [P, NRL, NJ], F32, kind="ExternalOutput")

    # const APs for activation biases (only 0.0/1.0 are pre-registered)
    def reg_const(v):
        v = _f(v)
        if (F32, v) not in nc.const_aps.aps:
            th = nc.alloc_sbuf_tensor(f"constap{len(nc.const_aps.aps)}", [128, 1], F32)
            nc.gpsimd.memset(th.ap(), v)
            nc.const_aps.aps[(F32, v)] = th.ap()

    reg_const(EPS)
    reg_const(-SHARP)
    for cc in consts:
        for j in range(3):
            reg_const(cc["rp"][j])
    nc.all_engine_barrier()

    live = [k for k in range(N_SQ) if rects[k] is not None]

    with tile.TileContext(nc) as tc, ExitStack() as es:
        V = nc.vector
        S = nc.scalar
        persist = es.enter_context(tc.tile_pool(name="persist", bufs=1))

        # ---- shared loads & per-core shared prep ----
        rd = persist.tile([P, 3, NRL, NJ], F32, name="rd")
        nc.sync.dma_start(rd[:, :, :, :], rd_dram.ap())

        rdsq = persist.tile([P, 3, NRL, NJ], F32, name="rdsq")
        S.activation(rdsq[:, :, :, :], rd[:, :, :, :], AF.Square)
        nd2 = persist.tile([P, NRL, NJ], F32, name="nd2")
        V.tensor_tensor(nd2[:, :, :], rdsq[:, 0, :, :], rdsq[:, 1, :, :], OP.add)
        V.tensor_tensor(nd2[:, :, :], nd2[:, :, :], rdsq[:, 2, :, :], OP.add)
        nd = persist.tile([P, NRL, NJ], F32, name="nd")
        S.activation(nd[:, :, :], nd2[:, :, :], AF.Ln)
        S.activation(nd[:, :, :], nd[:, :, :], AF.Exp, scale=0.5)

        dmin = persist.tile([P, NRL, NJ], F32, name="dmin")
        V.memset(dmin[:, :, :], FAR)

        # persistent per-SQ results for phase 2 (sized per rect)
        FF, HG, DTT = {}, {}, {}
        for k in live:
            lr0, nr, j0, nj = rects[k]
            X = nr * nj
            FF[k] = persist.tile([P, NS + 1, X], F32, name=f"FF{k}")
            HG[k] = persist.tile([P, X], F32, name=f"HG{k}")
            DTT[k] = persist.tile([P, 2, X], F32, name=f"DTT_{k}")

        # ---------------- phase 1: per-SQ F chains (ln/exp table set) -------
        with tc.tile_pool(name="p1", bufs=3) as pool:
            for k in live:
                cc = consts[k]
                E = V
                lr0, nr, j0, nj = rects[k]
                X = nr * nj
                M1, tcv, rp, rps, s = cc["M1"], cc["tc"], cc["rp"], cc["rps"], cc["s"]

                def r4(ap2):   # [P, X] compact view -> [P, nr, nj]
                    return ap2.rearrange("p (a b) -> p a b", b=nj)

                # compact copies of the rect slice of rd (and ||d||)
                rdc = pool.tile([P, 3, X], F32, tag="rdc")
                E.tensor_copy(rdc[:, :, :].rearrange("p c (a b) -> p c a b", b=nj),
                              rd[:, :, lr0:lr0 + nr, j0:j0 + nj])
                ndc = pool.tile([P, X], F32, tag="ndc")
                E.tensor_copy(r4(ndc[:, :]), nd[:, lr0:lr0 + nr, j0:j0 + nj])

                u = pool.tile([P, 3, X], F32, tag="u")
                for j in range(3):
                    E.tensor_scalar(u[:, j, :], rdc[:, 0, :], _f(M1[j, 0]), None, OP.mult)
                    E.scalar_tensor_tensor(u[:, j, :], rdc[:, 1, :], _f(M1[j, 1]), u[:, j, :], OP.mult, OP.add)
                    E.scalar_tensor_tensor(u[:, j, :], rdc[:, 2, :], _f(M1[j, 2]), u[:, j, :], OP.mult, OP.add)

                usq = pool.tile([P, 3, X], F32, tag="usq")
                E.tensor_tensor(usq[:, :, :], u[:, :, :], u[:, :, :], OP.mult)
                nu2 = pool.tile([P, X], F32, tag="nu2")
                E.tensor_tensor(nu2[:], usq[:, 0, :], usq[:, 1, :], OP.add)
                E.tensor_tensor(nu2[:], nu2[:], usq[:, 2, :], OP.add)

                rinv = pool.tile([P, X], F32, tag="rinv")
                S.activation(rinv[:], nu2[:], AF.Ln)
                S.activation(rinv[:], rinv[:], AF.Exp, scale=-0.5)

                td = pool.tile([P, 3, X], F32, tag="td")
                for j in range(3):
                    E.tensor_tensor(td[:, j, :], u[:, j, :], rinv[:], OP.mult)

                d1 = pool.tile([P, X], F32, tag="d1")
                E.tensor_scalar(d1[:], td[:, 0, :], _f(tcv[0]), None, OP.mult)
                E.scalar_tensor_tensor(d1[:], td[:, 1, :], _f(tcv[1]), d1[:], OP.mult, OP.add)
                E.scalar_tensor_tensor(d1[:], td[:, 2, :], _f(tcv[2]), d1[:], OP.mult, OP.add)
                proj = pool.tile([P, X], F32, tag="proj")
                S.activation(proj[:], d1[:], AF.Abs)

                cen = pool.tile([P, 3, X], F32, tag="cen")
                for j in range(3):
                    E.tensor_tensor(cen[:, j, :], proj[:], td[:, j, :], OP.mult)
                    E.tensor_scalar(cen[:, j, :], cen[:, j, :], _f(tcv[j]), None, OP.add)

                csq = pool.tile([P, 3, X], F32, tag="usq")
                E.tensor_tensor(csq[:, :, :], cen[:, :, :], cen[:, :, :], OP.mult)
                m3 = pool.tile([P, X], F32, tag="m3")
                E.tensor_tensor(m3[:], csq[:, 0, :], csq[:, 1, :], OP.add)
                E.tensor_tensor(m3[:], m3[:], csq[:, 2, :], OP.add)
                # m3 = 3 - dist^2 ; mask = m3 > 0 ; hclsq = max(m3, 1e-12)
                E.tensor_scalar(m3[:], m3[:], -1.0, 3.0, OP.mult, OP.add)
                E.tensor_scalar(m3[:], m3[:], 1e-12, None, OP.max)

                hcl = pool.tile([P, X], F32, tag="hcl")
                S.activation(hcl[:], m3[:], AF.Ln)
                S.activation(hcl[:], hcl[:], AF.Exp, scale=0.5)

                # hg = hcl * ||d|| * rinv
                E.tensor_tensor(HG[k][:], ndc[:], rinv[:], OP.mult)
                E.tensor_tensor(HG[k][:], HG[k][:], hcl[:], OP.mult)

                htd = pool.tile([P, 3, X], F32, tag="htd")
                for j in range(3):
                    E.tensor_tensor(htd[:, j, :], hcl[:], td[:, j, :], OP.mult)

                # PL slots 0..9: cen + t_s*htd ; slot 10: 1.5*u - rp/s
                PL = pool.tile([P, NS + 1, 3, X], F32, tag="PL", bufs=3)
                for si in range(NS):
                    E.scalar_tensor_tensor(PL[:, si, :, :], htd[:, :, :], _f(t[si]),
                                           cen[:, :, :], OP.mult, OP.add)
                for j in range(3):
                    E.tensor_scalar(PL[:, NS, j, :], u[:, j, :], 1.5, _f(-rps[j]),
                                    OP.mult, OP.add)

                # dt0 = ||PL0*s + rp|| ; dt10 = ||(PL10-PL9)*s||
                q3 = pool.tile([P, 3, X], F32, tag="q3")
                for j in range(3):
                    S.activation(q3[:, j, :], PL[:, 0, j, :], AF.Square,
                                 bias=_f(rp[j]), scale=_f(s[j]))
                dtt = DTT[k]
                E.tensor_tensor(dtt[:, 0, :], q3[:, 0, :], q3[:, 1, :], OP.add)
                E.tensor_tensor(dtt[:, 0, :], dtt[:, 0, :], q3[:, 2, :], OP.add)

                df = pool.tile([P, 3, X], F32, tag="q3b")
                E.tensor_tensor(df[:, :, :], PL[:, NS, :, :], PL[:, NS - 1, :, :], OP.subtract)
                for j in range(3):
                    S.activation(df[:, j, :], df[:, j, :], AF.Square, scale=_f(s[j]))
                E.tensor_tensor(dtt[:, 1, :], df[:, 0, :], df[:, 1, :], OP.add)
                E.tensor_tensor(dtt[:, 1, :], dtt[:, 1, :], df[:, 2, :], OP.add)
                S.activation(dtt[:, :, :], dtt[:, :, :], AF.Ln)
                S.activation(dtt[:, :, :], dtt[:, :, :], AF.Exp, scale=0.5)

                # F chain, in place over PL
                flat = PL[:, :, :, :]
                S.activation(flat, flat, AF.Abs)                       # |PL|
                S.activation(flat, flat, AF.Ln, bias=_f(EPS))          # ln(|PL|+eps)
                S.activation(PL[:, :, 0:2, :], PL[:, :, 0:2, :], AF.Exp,
                             scale=_f(cc["c1"]))                       # u,v
                E.tensor_tensor(PL[:, :, 0, :], PL[:, :, 0, :], PL[:, :, 1, :], OP.add)
                S.activation(PL[:, :, 0, :], PL[:, :, 0, :], AF.Ln)
                S.activation(PL[:, :, 0, :], PL[:, :, 0, :], AF.Exp, scale=_f(cc["c2"]))
                S.activation(PL[:, :, 2, :], PL[:, :, 2, :], AF.Exp, scale=_f(cc["c3"]))
                E.tensor_tensor(PL[:, :, 0, :], PL[:, :, 0, :], PL[:, :, 2, :], OP.add)
                S.activation(PL[:, :, 0, :], PL[:, :, 0, :], AF.Ln)
                S.activation(FF[k][:, :, :], PL[:, :, 0, :], AF.Exp, scale=_f(cc["e1"]))

                # ---- occupancy/visibility/depth (same ln/exp table set) ----
                # occ = sigmoid(1000*(1-F)) = 1/(1 + e^(1000F-1000)); F clamped
                # at 1.088 so e^x <= 1.65e38 (sigma there is 6e-39 ~ 0).
                occ = pool.tile([P, NS + 1, X], F32, tag="occ", bufs=3)
                E.tensor_scalar(FF[k][:, :, :], FF[k][:, :, :], 1.088, None, OP.min)
                S.activation(occ[:, :, :], FF[k][:, :, :], AF.Exp,
                             scale=SHARP, bias=-SHARP)
                S.activation(occ[:, :, :], occ[:, :, :], AF.Identity, bias=1.0)
                rscr = pool.tile([P, NS + 1, X], F32, tag="rscr")
                E.reciprocal_approx_fast(rscr[:, :, :], occ[:, :, :])

                cum = pool.tile([P, NS + 1, X], F32, tag="cum", bufs=3)
                E.tensor_scalar(cum[:, 0, :], rscr[:, 0, :], _f(cc["occ0"]), None, OP.add)
                for si in range(1, NS + 1):
                    E.tensor_tensor(cum[:, si, :], cum[:, si - 1, :], rscr[:, si, :], OP.add)
                S.activation(cum[:, :, :], cum[:, :, :], AF.Exp, scale=-TAU)  # v_1..v_11

                acc = pool.tile([P, X], F32, tag="acc")
                E.tensor_scalar(acc[:], cum[:, 0, :], _f(beta[1]), None, OP.mult)
                for si in range(1, NS):
                    E.scalar_tensor_tensor(acc[:], cum[:, si, :], _f(beta[si + 1]),
                                           acc[:], OP.mult, OP.add)
                E.tensor_tensor(acc[:], acc[:], HG[k][:], OP.mult)

                b1 = pool.tile([P, X], F32, tag="b1")
                E.tensor_scalar(b1[:], cum[:, 0, :], 0.5, _f(0.5 * cc["vis0"]),
                                OP.mult, OP.add)
                E.tensor_tensor(b1[:], b1[:], DTT[k][:, 0, :], OP.mult)
                E.tensor_tensor(acc[:], acc[:], b1[:], OP.add)

                b2 = pool.tile([P, X], F32, tag="b2")
                E.tensor_tensor(b2[:], cum[:, NS - 1, :], cum[:, NS, :], OP.add)
                E.scalar_tensor_tensor(b2[:], b2[:], 0.5, DTT[k][:, 1, :], OP.mult, OP.mult)
                E.tensor_tensor(acc[:], acc[:], b2[:], OP.add)

                # masked-out rect pixels integrate to 1.5 +- 1e-6 == FAR
                # (F > 1 strictly outside the bounding sphere => vis == 1,
                # and the sample polyline is monotone on the ray), so the
                # explicit mask/select is unnecessary: min() absorbs them.
                dv = dmin[:, lr0:lr0 + nr, j0:j0 + nj]
                V.tensor_tensor(dv, dv, acc[:].rearrange("p (a b) -> p a b", b=nj),
                                OP.min)


        nc.sync.dma_start(out_dram.ap(), dmin[:, :, :])

    # Pre-place the two ACT table loads (natural_log_exp for phase 1,
    # exp_and_others for phase 2/tanh) so bacc's fixpoint inserts none.
    # (CoreSim can't handle the hand-inserted loads; act_loads=False skips.)
    if not act_loads:
        nc.compile()
        return nc
    from concourse.hw_specs import get_activation_tables
    names = list(get_activation_tables(nc.m.arch).keys())
    id_nle = names.index("natural_log_exp_and_others")

    def make_load(set_id):
        ins = mybir.InstLoadActFuncSet(
            name=nc.get_next_instruction_name(), act_func_set_id=set_id,
            ins=[], outs=[])
        ins.engine = nc.scalar.engine
        return ins

    for blk in nc.main_func.blocks:
        il = blk.instructions
        first_act = next((i for i, x in enumerate(il)
                          if isinstance(x, mybir.InstActivation)), None)
        if first_act is None:
            continue
        il.insert(first_act, make_load(id_nle))

    nc.compile()
    return nc


def _shard_rays(rays_d):
    """-> per-core arrays [128, 3, 45, 5]; core c owns rows 8*lr+c."""
    rd = np.asarray(rays_d, np.float32)
    out = []
    for c in range(N_CORES):
        sub = rd[c::N_CORES]                         # (45, 640, 3)
        arr = sub.reshape(NRL, NJ, 128, 3).transpose(2, 3, 0, 1)
        out.append(np.ascontiguousarray(arr))        # (128, 3, 45, 5)
    return out


def _unshard(outs):
    """outs: list of 8 arrays [128, 45, 5] -> (360, 640)."""
    full = np.empty((HS, WS), np.float32)
    for c in range(N_CORES):
        full[c::N_CORES] = outs[c].transpose(1, 2, 0).reshape(NRL, WS)
    return full


def kernel(sq_poses, sq_params, rays_d, rays_o, t, **run_kwargs):
    consts, tv, beta = _host_consts(sq_poses, sq_params, rays_o, t)
    rects = _host_rects(consts, rays_d)
    nc = build_program(consts, tv, beta, rects)
    planes = _shard_rays(rays_d)
    in_maps = [{"rdin": planes[c]} for c in range(N_CORES)]
    res = run_bass_kernel_spmd(nc, in_maps, core_ids=list(range(N_CORES)), **run_kwargs)
    outs = [res.results[c]["depth"] for c in range(N_CORES)]
    out = _unshard(outs).astype(np.float32)
    kernel.last_result = res
    return out


kernel.last_result = None
